# revision 1
# baseline (speedup 1.0000x reference)
"""BitNet MLP (nn_BitNetMLP_19421842112750) — TRN2 Bass kernel, 8-core
data-parallel over tokens.

Per core (T=1024 tokens of the 8192 total):
  G = x @ Wg_tern.T ; U = x @ Wu_tern.T          (ternary weights, scales folded)
  h = relu(G)^2 * U                               (= hidden_ref / c, c = gs^2*us)
  hn = h * norm_w
  var' = (sum_i hn^2 / nw^2) / I + eps/c^2        (= (var_ref + eps)/c^2)
  out = (hn @ Wd_tern.T) * ds * rsqrt(var')       (== reference output exactly)

The 2-bit weights are unpacked on device: a [128, N] tile of packed bytes is
expanded by 4 DVE tensor_scalar ops ((b >> s) & 3, immediate scalars) plus one
ACT copy (bias=-1, u8 -> bf16). Each shift-op's output partition equals its
input partition, which scrambles the contraction order; since matmul
contraction is order-invariant, the scramble is absorbed by host-side row
permutation of x.T (pi_H) and column permutation of gate/up packed + norm_w
(pi_I). down_packed.T's natural layout matches pi_I exactly (incl. a
256-element ragged tail handled with 64-partition matmul chunks).

Device layouts:
  xTp   [H, T]   f32  rows: h = pi_H(r)        (per-core x shard, transposed)
  gpTp  [H/4, I] u8   cols: i = pi_I(c)        (same for upTp)
  dpT   [I/4, H] u8   natural (= down_packed.T)
  nw_p  [I]      f32  nw_p[r] = norm_w[pi_I(r)]
  outT  [H, T]   f32  natural h rows (host transposes back)
"""

import sys

sys.path.insert(0, "/opt/trn_rl_repo")
from contextlib import ExitStack

import numpy as np

import concourse.bass as bass
import concourse.tile as tile
from concourse import bacc, mybir

F32 = mybir.dt.float32
BF16 = mybir.dt.bfloat16
U8 = mybir.dt.uint8
U32 = mybir.dt.uint32
SWAR_MASK = 0x03030303
AOT = mybir.AluOpType
ACTF = mybir.ActivationFunctionType
RMS_EPS = 1e-6

N_CORES = 8
FULL_B, FULL_S, FULL_H, FULL_I = 4, 2048, 2560, 6912


# ---------------------------------------------------------------- permutations
def perm_H(n):
    """SBUF row r -> original h index. Groups of 512 (4 chunks x 128)."""
    assert n % 512 == 0
    r = np.arange(n)
    c, p = r // 128, r % 128
    return 512 * (c // 4) + 128 * (p // 32) + 32 * (c % 4) + (p % 32)


def perm_I(n):
    """hidden SBUF row r -> original i index. Full 512-groups, then a
    256-tail (two 128-tiles, each split into 64-partition halves)."""
    r = np.arange(n)
    c, p = r // 128, r % 128
    out = 512 * (c // 4) + 128 * (p // 32) + 32 * (c % 4) + (p % 32)
    n_full = (n // 512) * 512
    if n_full != n:
        assert n - n_full == 256, "tail must be exactly 256"
        off = r[n_full:] - n_full
        tile_off, p2 = off // 128, off % 128
        s, q, j = p2 // 64, (p2 % 64) // 32, p2 % 32
        k = 2 * tile_off + s
        out[n_full:] = n_full + 128 * q + 32 * k + j
    return out


# ---------------------------------------------------------------- the program
def build_program(T, H, I, TC=512, W_I=128, HBAND=4, stage=4):
    """Build the single-core Bass program (SPMD-identical across cores)."""
    assert H % 512 == 0 and T % TC == 0 and I % W_I == 0 and W_I % 128 == 0
    assert TC == 512  # PSUM zero-region granularity
    NH = H // 128  # h chunks (gate/up contraction)
    NI = I // 128  # i tiles (hidden rows)
    NT = T // TC  # t chunks
    NB = I // W_I  # gate/up i bands
    TPB = W_I // 128  # i tiles per band
    NHB = (H // 128) // HBAND  # down h bands
    IB = I // 4  # down packed rows
    NC_FULL = IB // 128  # full down C-groups
    C_TAIL = IB % 128  # 0 or 64
    NGH = H // 512  # gate/up stage groups

    nc = bacc.Bacc("TRN2", target_bir_lowering=False, debug=False)

    xTp = nc.dram_tensor("xTp", [H, T], F32, kind="ExternalInput").ap()
    gpTp = nc.dram_tensor("gpTp", [H // 4, I], U8, kind="ExternalInput").ap()
    upTp = nc.dram_tensor("upTp", [H // 4, I], U8, kind="ExternalInput").ap()
    dpT = nc.dram_tensor("dpT", [IB, H], U8, kind="ExternalInput").ap()
    nw = nc.dram_tensor("nw_p", [I], F32, kind="ExternalInput").ap()
    eps_in = nc.dram_tensor("eps_in", [1], F32, kind="ExternalInput").ap()
    ds2_in = nc.dram_tensor("ds2_in", [1], F32, kind="ExternalInput").ap()
    outT = nc.dram_tensor("outT", [H, T], F32, kind="ExternalOutput").ap()

    with tile.TileContext(nc) as tc, ExitStack() as top:
        const = top.enter_context(tc.tile_pool(name="const", bufs=1))
        hn_pool = top.enter_context(tc.tile_pool(name="hn", bufs=1))
        hn_sb = hn_pool.tile([128, NI, T], BF16)

        # ---- host-derived scalars: eps' = eps/c^2 (c = gs^2*us), ds2 = ds^2
        eps_t = const.tile([1, 1], F32)
        ds2_t = const.tile([1, 1], F32)
        nc.sync.dma_start(eps_t[:], eps_in[None, :])
        nc.sync.dma_start(ds2_t[:], ds2_in[None, :])

        # ---- norm_w striped [128, NI] + 1/nw^2 in bf16
        nw_sb = const.tile([128, NI], F32)
        nc.sync.dma_start(nw_sb[:], nw.rearrange("(o p) -> p o", p=128))
        inw2 = const.tile([128, NI], F32)
        nc.vector.tensor_mul(inw2[:], nw_sb[:], nw_sb[:])
        nc.vector.reciprocal(inw2[:], inw2[:])
        inw2_bf = const.tile([128, NI], BF16)
        nc.vector.tensor_copy(inw2_bf[:], inw2[:])

        # ================= phase A: gate/up matmuls -> hn tiles ==============
        with tc.tile_pool(name="xT", bufs=1) as xT_pool:
            xT = xT_pool.tile([128, NH, T], BF16)
            with tc.tile_pool(name="xload", bufs=2) as xload:
                for c in range(NH):
                    xs = xload.tile([128, T], F32)
                    nc.sync.dma_start(xs[:], xTp[128 * c : 128 * (c + 1), :])
                    nc.vector.tensor_copy(xT[:, c, :], xs[:])
            if stage < 2:
                NB_eff = 0
            else:
                NB_eff = NB
            with (
                tc.tile_pool(name="wband", bufs=2) as wband,
                tc.tile_pool(name="ustage", bufs=3) as ustage,
                tc.tile_pool(name="ush", bufs=2) as ush,
                tc.tile_pool(name="psA", bufs=2, space="PSUM") as psA,
                tc.tile_pool(name="ract", bufs=2) as ract,
            ):
              for band in range(NB_eff):
                  i0 = band * W_I
                  wg_band = wband.tile([128, NH, W_I], BF16, tag="wg")
                  wu_band = wband.tile([128, NH, W_I], BF16, tag="wu")
                  for mat_ap, wt in ((gpTp, wg_band), (upTp, wu_band)):
                      for B in range(NGH):
                          st = ustage.tile([128, W_I], U8)
                          nc.sync.dma_start(
                              st[:], mat_ap[128 * B : 128 * (B + 1), i0 : i0 + W_I]
                          )
                          sh4 = ush.tile([128, 4, W_I], U8)
                          stw = st[:].bitcast(U32)
                          for k in range(4):
                              nc.vector.tensor_scalar(
                                  sh4[:, k, :].bitcast(U32),
                                  stw,
                                  6 - 2 * k,
                                  SWAR_MASK,
                                  AOT.logical_shift_right,
                                  AOT.bitwise_and,
                              )
                          nc.scalar.activation(
                              wt[:, 4 * B : 4 * B + 4, :], sh4[:], ACTF.Copy, bias=-1.0
                          )
                  for itl in range(TPB):
                      it = band * TPB + itl
                      isl = slice(128 * itl, 128 * (itl + 1))
                      pg = psA.tile([128, T], F32, tag="pg")
                      pu = psA.tile([128, T], F32, tag="pu")
                      for c in range(NH):
                          for t in range(NT):
                              tsl = slice(TC * t, TC * (t + 1))
                              nc.tensor.matmul(
                                  pg[:, tsl],
                                  wg_band[:, c, isl],
                                  xT[:, c, tsl],
                                  start=(c == 0),
                                  stop=(c == NH - 1),
                              )
                          for t in range(NT):
                              tsl = slice(TC * t, TC * (t + 1))
                              nc.tensor.matmul(
                                  pu[:, tsl],
                                  wu_band[:, c, isl],
                                  xT[:, c, tsl],
                                  start=(c == 0),
                                  stop=(c == NH - 1),
                              )
                      r = ract.tile([128, T], BF16, tag="r")
                      nc.scalar.activation(r[:], pg[:], ACTF.Relu)
                      nc.vector.tensor_mul(r[:], r[:], r[:])
                      nc.vector.tensor_mul(r[:], r[:], pu[:])
                      nc.vector.tensor_scalar(
                          hn_sb[:, it, :], r[:], nw_sb[:, it : it + 1], None, AOT.mult
                      )

        # ================= phase A2: variance -> broadcast ds*rstd ==========
        if stage >= 3:
          with (
            tc.tile_pool(name="psV", bufs=1, space="PSUM") as psV,
            tc.tile_pool(name="h2p", bufs=2) as h2p,
            tc.tile_pool(name="vmisc", bufs=1) as vmisc,
          ):
            pv = psV.tile([1, T], F32)
            for it in range(NI):
                h2 = h2p.tile([128, T], BF16)
                nc.vector.tensor_mul(h2[:], hn_sb[:, it, :], hn_sb[:, it, :])
                for t in range(NT):
                    tsl = slice(TC * t, TC * (t + 1))
                    nc.tensor.matmul(
                        pv[:, tsl],
                        inw2_bf[:, it : it + 1],
                        h2[:, tsl],
                        start=(it == 0),
                        stop=(it == NI - 1),
                    )
            var_sb = vmisc.tile([1, T], F32)
            nc.scalar.activation(
                var_sb[:], pv[:], ACTF.Identity, bias=eps_t[:], scale=1.0 / I
            )
            rv = vmisc.tile([1, T], F32)
            nc.vector.reciprocal(rv[:], var_sb[:])
            rstd = vmisc.tile([1, T], F32)
            nc.scalar.activation(rstd[:], rv[:], ACTF.Sqrt, scale=ds2_t[:])
            # broadcast rstd to all 128 partitions via K=1 matmul with ones
            ones_row = vmisc.tile([1, 128], F32)
            nc.vector.memset(ones_row[:], 1.0)
            ps_bc = psV.tile([128, T], F32, name="ps_bc")
            for t in range(NT):
                tsl = slice(TC * t, TC * (t + 1))
                nc.tensor.matmul(
                    ps_bc[:, tsl], ones_row[:], rstd[:, tsl], start=True, stop=True
                )
            rstd_bc = const.tile([128, T], F32)
            nc.scalar.activation(rstd_bc[:], ps_bc[:], ACTF.Copy)

        # ================= phase B: down matmul =============================
        if stage >= 4:
          with (
            tc.tile_pool(name="dstage", bufs=3) as dstage,
            tc.tile_pool(name="dsh", bufs=2) as dsh,
            tc.tile_pool(name="wd", bufs=2) as wd,
            tc.tile_pool(name="psB", bufs=1, space="PSUM") as psB,
            tc.tile_pool(name="outp", bufs=3) as outp,
          ):
            n_cgrp = NC_FULL + (1 if C_TAIL else 0)
            for hb in range(NHB):
                h0 = 128 * HBAND * hb
                po = [
                    [
                        psB.tile([128, TC], F32, tag=f"po_{ht}_{t}", name=f"po_{ht}_{t}")
                        for t in range(NT)
                    ]
                    for ht in range(HBAND)
                ]
                for C in range(n_cgrp):
                    tail = C >= NC_FULL
                    st = dstage.tile([128, HBAND * 128], U8, tag="dst")
                    if not tail:
                        nc.sync.dma_start(
                            st[:], dpT[128 * C : 128 * (C + 1), h0 : h0 + HBAND * 128]
                        )
                    else:
                        # replicate the 64 tail rows into both partition halves
                        src = dpT[128 * C : 128 * C + C_TAIL, h0 : h0 + HBAND * 128]
                        nc.sync.dma_start(st[:C_TAIL], src)
                        nc.sync.dma_start(st[C_TAIL : 2 * C_TAIL], src)
                    sh4 = dsh.tile([128, 4, HBAND * 128], U8)
                    stw = st[:].bitcast(U32)
                    for k in range(4):
                        nc.vector.tensor_scalar(
                            sh4[:, k, :].bitcast(U32),
                            stw,
                            6 - 2 * k,
                            SWAR_MASK,
                            AOT.logical_shift_right,
                            AOT.bitwise_and,
                        )
                    wdt = wd.tile([128, 4, HBAND * 128], BF16)
                    nc.scalar.activation(wdt[:], sh4[:], ACTF.Copy, bias=-1.0)
                    for k in range(4):
                        if not tail:
                            rhs_tile, rp0, rows = 4 * C + k, 0, 128
                        else:
                            rhs_tile = 4 * C + k // 2
                            rp0, rows = C_TAIL * (k % 2), C_TAIL
                        for ht in range(HBAND):
                            hsl = slice(128 * ht, 128 * (ht + 1))
                            for t in range(NT):
                                tsl = slice(TC * t, TC * (t + 1))
                                nc.tensor.matmul(
                                    po[ht][t][:, :],
                                    wdt[rp0 : rp0 + rows, k, hsl],
                                    hn_sb[rp0 : rp0 + rows, rhs_tile, tsl],
                                    start=(C == 0 and k == 0),
                                    stop=(C == n_cgrp - 1 and k == 3),
                                )
                for ht in range(HBAND):
                    for t in range(NT):
                        tsl = slice(TC * t, TC * (t + 1))
                        ot = outp.tile([128, TC], F32)
                        nc.vector.tensor_mul(ot[:], po[ht][t][:], rstd_bc[:, tsl])
                        nc.sync.dma_start(
                            outT[h0 + 128 * ht : h0 + 128 * (ht + 1), tsl], ot[:]
                        )

    nc.compile()
    return nc


# ------------------------------------------------------------- host-side prep
def prep_inputs(x, gate_packed, gate_scale, up_packed, up_scale, down_packed,
                down_scale, norm_w, n_cores):
    """Full inputs -> per-core in_maps in device layout (slicing + relayout)."""
    B, S, H = x.shape
    I = norm_w.shape[0]
    T_full = B * S
    T = T_full // n_cores
    piH = perm_H(H)
    piI = perm_I(I)

    gpTp = np.ascontiguousarray(gate_packed[piI].T)  # [H/4, I]
    upTp = np.ascontiguousarray(up_packed[piI].T)
    dpT = np.ascontiguousarray(down_packed.T)  # [I/4, H]
    nw_p = np.ascontiguousarray(norm_w[piI]).astype(np.float32)
    xf = x.reshape(T_full, H)

    gs_v = float(np.asarray(gate_scale).reshape(-1)[0])
    us_v = float(np.asarray(up_scale).reshape(-1)[0])
    ds_v = float(np.asarray(down_scale).reshape(-1)[0])
    c = gs_v * gs_v * us_v
    eps_p = np.asarray([RMS_EPS / (c * c)], np.float32)
    ds2 = np.asarray([ds_v * ds_v], np.float32)

    in_maps = []
    for cid in range(n_cores):
        xs = xf[cid * T : (cid + 1) * T]
        xTp = np.ascontiguousarray(xs.T[piH])  # [H, T]
        in_maps.append(
            {
                "xTp": xTp,
                "gpTp": gpTp,
                "upTp": upTp,
                "dpT": dpT,
                "nw_p": nw_p,
                "eps_in": eps_p,
                "ds2_in": ds2,
            }
        )
    return in_maps


def assemble_output(results, B, S, H):
    """Per-core outT [H, T] -> full [B, S, H]."""
    outs = [np.asarray(r["outT"]).T for r in results]  # each [T, H]
    return np.ascontiguousarray(np.concatenate(outs, axis=0).reshape(B, S, H))


# ---------------------------------------------------------------- entry point
_CACHED = {}


def _get_program():
    if "nc" not in _CACHED:
        T = FULL_B * FULL_S // N_CORES
        _CACHED["nc"] = build_program(T, FULL_H, FULL_I)
    return _CACHED["nc"]


def kernel(x, gate_packed, gate_scale, up_packed, up_scale, down_packed,
           down_scale, norm_w, _trace=False):
    from concourse.bass_utils import run_bass_kernel_spmd

    x = np.asarray(x, np.float32)
    gate_packed = np.asarray(gate_packed, np.uint8)
    up_packed = np.asarray(up_packed, np.uint8)
    down_packed = np.asarray(down_packed, np.uint8)
    norm_w = np.asarray(norm_w, np.float32)

    B, S, H = x.shape
    in_maps = prep_inputs(
        x, gate_packed, gate_scale, up_packed, up_scale, down_packed,
        down_scale, norm_w, N_CORES,
    )
    nc = _get_program()
    res = run_bass_kernel_spmd(nc, in_maps, list(range(N_CORES)), trace=_trace)
    out = assemble_output(res.results, B, S, H)
    if _trace:
        kernel.last_results = res
    return out



# revision 8
# speedup vs baseline: 1.0276x; 1.0276x over previous
"""BitNet MLP (nn_BitNetMLP_19421842112750) — TRN2 Bass kernel, 8-core
data-parallel over tokens, fp8 DoubleRow matmuls with exact hi/lo splitting.

Per core (T=1024 tokens of the 8192 total):
  G = x @ Wg_tern.T ; U = x @ Wu_tern.T
  h = relu(G)^2 * U
  var = (sum_i h^2)*A + eps ; rstd = sqrt(D / var)
  out = ((h*nw*k) @ Wd_tern.T) * rstd

All three matmuls run as fp8e4 DoubleRow at 2x PE throughput with NO fp8
quantization error in the weights (ternary {-1,0,1,2} is exact in e4m3) and
~9-bit effective mantissa on activations: each activation value v is split
into (hi, lo) = (e4m3(v), e4m3(v - hi)) and the stationary weight is
broadcast (stride-0) along the DoubleRow pair dim, so each PE cell computes
w*hi + w*lo = w*v. x is split on the host; the hidden layer is split on
DVE. Scales gs/us/ds/s_x/k_h are folded into host-side constants A, D and
into nw.

Weight unpack (2-bit -> fp8e4) runs on device: DVE SWAR shift/mask ops then
convert(+(-1)) split between ACT and DVE. Permutations pi_H / pi_I absorb
the unpack partition scramble exactly as in the bf16 version.

Device layouts:
  x8p  [H, 2, T]  fp8  rows: h = pi_H(r), pair dim = (hi, lo)
  gpTp [H/4, I]   u8   cols: i = pi_I(c)   (same for upTp)
  dpT  [I/4, H]   u8   natural (= down_packed.T)
  nwk  [I]        f32  nwk[r] = norm_w[pi_I(r)] * k_h
  outT [H, T]     bf16 natural h rows (host transposes back)
"""

import sys

sys.path.insert(0, "/opt/trn_rl_repo")
from contextlib import ExitStack

import numpy as np
import ml_dtypes

import concourse.bass as bass
import concourse.tile as tile
from concourse import bacc, mybir

F32 = mybir.dt.float32
BF16 = mybir.dt.bfloat16
U8 = mybir.dt.uint8
U32 = mybir.dt.uint32
FP8 = mybir.dt.float8e4
SWAR_MASK = 0x03030303
AOT = mybir.AluOpType
ACTF = mybir.ActivationFunctionType
DR = mybir.MatmulPerfMode.DoubleRow
E4M3 = ml_dtypes.float8_e4m3fn
RMS_EPS = 1e-6

N_CORES = 8
FULL_B, FULL_S, FULL_H, FULL_I = 4, 2048, 2560, 6912


# ---------------------------------------------------------------- permutations
def perm_H(n):
    """SBUF row r -> original h index. Groups of 512 (4 chunks x 128)."""
    assert n % 512 == 0
    r = np.arange(n)
    c, p = r // 128, r % 128
    return 512 * (c // 4) + 128 * (p // 32) + 32 * (c % 4) + (p % 32)


def perm_I(n):
    """hidden SBUF row r -> original i index. Full 512-groups, then a
    256-tail (two 128-tiles, each split into 64-partition halves)."""
    r = np.arange(n)
    c, p = r // 128, r % 128
    out = 512 * (c // 4) + 128 * (p // 32) + 32 * (c % 4) + (p % 32)
    n_full = (n // 512) * 512
    if n_full != n:
        assert n - n_full == 256, "tail must be exactly 256"
        off = r[n_full:] - n_full
        tile_off, p2 = off // 128, off % 128
        s, q, j = p2 // 64, (p2 % 64) // 32, p2 % 32
        k = 2 * tile_off + s
        out[n_full:] = n_full + 128 * q + 32 * k + j
    return out


# ---------------------------------------------------------------- the program
def build_program(T, H, I):
    """Build the single-core Bass program (SPMD-identical across cores)."""
    NH = H // 128          # x chunks / gate-up contraction chunks (20)
    NGH = H // 512         # packed row groups per gate/up band (5)
    NI = I // 128          # hidden i-tiles (54)
    NB = NI                # gate/up bands (W_I = 128)
    TC = 512
    NT = T // TC           # 2
    IB = I // 4            # down packed rows (1728)
    NC_FULL = IB // 128    # 13
    C_TAIL = IB % 128      # 64
    # tail re-packed host-side into one extra full 128-row block whose
    # shift-0/1 fields hold the last 256 i-values (shift-2/3 fields = 0)
    IB_PAD = (NC_FULL + 1) * 128 if C_TAIL else IB
    assert (NH - 4) % 4 == 0
    HBANDS = [2] + [4] * ((NH - 4) // 4) + [2]
    assert sum(HBANDS) == NH

    nc = bacc.Bacc("TRN2", target_bir_lowering=False, debug=False)

    x8p = nc.dram_tensor("x8p", [H, 2, T], FP8, kind="ExternalInput").ap()
    gpTp = nc.dram_tensor("gpTp", [H // 4, I], U8, kind="ExternalInput").ap()
    upTp = nc.dram_tensor("upTp", [H // 4, I], U8, kind="ExternalInput").ap()
    dpT = nc.dram_tensor("dpT", [IB_PAD, H], U8, kind="ExternalInput").ap()
    nwk = nc.dram_tensor("nwk", [I], F32, kind="ExternalInput").ap()
    a_in = nc.dram_tensor("a_in", [1], F32, kind="ExternalInput").ap()
    d_in = nc.dram_tensor("d_in", [1], F32, kind="ExternalInput").ap()
    outT = nc.dram_tensor("outT", [H, T], BF16, kind="ExternalOutput").ap()

    def bc2(w):  # stationary [K, M] -> [K, 2, M] stride-0 DoubleRow pair
        return w.unsqueeze(1).broadcast_to([w.shape[0], 2, w.shape[1]])

    with tile.TileContext(nc) as tc, ExitStack() as top:
        const = top.enter_context(tc.tile_pool(name="const", bufs=1))
        hpool = top.enter_context(tc.tile_pool(name="h8", bufs=1))
        h8 = hpool.tile([128, NI, 2, T], FP8)

        # ---- x chunks first so chunk 0 lands ASAP
        xpool = top.enter_context(tc.tile_pool(name="xT", bufs=1))
        xts = []
        for c in range(NH):
            xt = xpool.tile([128, 2, T], FP8, name=f"x{c}")
            nc.sync.dma_start(xt[:], x8p[128 * c : 128 * (c + 1), :, :])
            xts.append(xt)

        # ---- constants
        a_t = const.tile([1, 1], F32)
        d_t = const.tile([1, 1], F32)
        nc.sync.dma_start(a_t[:], a_in[None, :])
        nc.sync.dma_start(d_t[:], d_in[None, :])
        nw_sb = const.tile([128, NI], F32)
        nc.sync.dma_start(nw_sb[:], nwk.rearrange("(o p) -> p o", p=128))
        s_acc = const.tile([128, T], F32)
        nc.vector.memset(s_acc[:], 0.0)
        s_bf = const.tile([128, T], BF16)
        ones_col = const.tile([128, 1], BF16)
        nc.vector.memset(ones_col[:], 1.0)
        ones_row = const.tile([1, 128], F32)
        nc.vector.memset(ones_row[:], 1.0)
        rstd_bc = const.tile([128, T], F32)

        # ================= phase A: gate/up DoubleRow matmuls ===============
        with (
            tc.tile_pool(name="wstage", bufs=2) as wstage,
            tc.tile_pool(name="ush", bufs=2) as ush,
            tc.tile_pool(name="wband", bufs=2) as wband,
            tc.tile_pool(name="psA", bufs=2, space="PSUM") as psA,
            tc.tile_pool(name="ract", bufs=2) as ract,
        ):
            for it in range(NB):
                i0 = it * 128
                # stage both gate and up packed bytes: [128, 2, NGH*128]
                st = wstage.tile([128, 2, NGH * 128], U8, tag="st")
                for m, mat_ap in enumerate((gpTp, upTp)):
                    nc.sync.dma_start(
                        st[:, m, :].rearrange("p (b w) -> p b w", b=NGH),
                        mat_ap.rearrange("(b p) w -> p b w", p=128)[
                            :, :, i0 : i0 + 128
                        ],
                    )
                # SWAR 2-bit unpack: 4 shift/mask ops over both matrices
                sh = ush.tile([128, 4, 2, NGH * 128], U8, tag="sh")
                stw = st[:].bitcast(U32)
                for k in range(4):
                    nc.vector.tensor_scalar(
                        sh[:, k, :, :].bitcast(U32),
                        stw,
                        6 - 2 * k,
                        SWAR_MASK,
                        AOT.logical_shift_right,
                        AOT.bitwise_and,
                    )
                # convert to fp8 with -1 bias; k 0,1 on ACT / 2,3 on DVE
                wg8 = wband.tile([128, 4, NGH, 128], FP8, tag="wg")
                wu8 = wband.tile([128, 4, NGH, 128], FP8, tag="wu")
                for m, wt in ((0, wg8), (1, wu8)):
                    for k in range(4):
                        src = sh[:, k, m, :].rearrange("p (b w) -> p b w", b=NGH)
                        if k < 2:
                            nc.scalar.activation(
                                wt[:, k, :, :], src, ACTF.Copy, bias=-1.0
                            )
                        else:
                            nc.vector.tensor_scalar(
                                wt[:, k, :, :], src, -1.0, None, AOT.add
                            )
                # DoubleRow matmuls: G and U for this i-tile
                pg = psA.tile([128, T], F32, tag="pg")
                pu = psA.tile([128, T], F32, tag="pu")
                for c in range(NH):
                    Bq, k = divmod(c, 4)
                    for ps_t, wt in ((pg, wg8), (pu, wu8)):
                        lhsT = bc2(wt[:, k, Bq, :])
                        for t in range(NT):
                            tsl = slice(TC * t, TC * (t + 1))
                            nc.tensor.matmul(
                                ps_t[:, tsl],
                                lhsT,
                                xts[c][:, :, tsl],
                                start=(c == 0),
                                stop=(c == NH - 1),
                                perf_mode=DR,
                            )
                # r-stage: h = relu(G)^2 * U ; s_acc += h^2 ; h8 = split(h*nw)
                q = ract.tile([128, T], BF16, tag="q")
                h2t = ract.tile([128, T], F32, tag="h2t")
                nc.scalar.activation(q[:], pg[:], ACTF.Relu)
                nc.vector.tensor_mul(q[:], q[:], q[:])
                nc.vector.tensor_mul(q[:], q[:], pu[:])
                nc.vector.tensor_mul(h2t[:], q[:], q[:])
                nc.vector.tensor_tensor(s_acc[:], s_acc[:], h2t[:], AOT.add)
                nwc = nw_sb[:, it : it + 1]
                nc.vector.tensor_scalar(
                    h8[:, it, 0, :], q[:], nwc, None, AOT.mult
                )
                nc.vector.scalar_tensor_tensor(
                    h8[:, it, 1, :], q[:], nwc, h8[:, it, 0, :],
                    AOT.mult, AOT.subtract,
                )
                if it == NB - 1:
                    nc.vector.tensor_copy(s_bf[:], s_acc[:])

        # ================= phase B + variance finalization ==================
        n_cgrp = NC_FULL + (1 if C_TAIL else 0)
        vpool = top.enter_context(tc.tile_pool(name="vmisc", bufs=1))
        var_sb = vpool.tile([1, T], F32)
        rv = vpool.tile([1, T], F32)
        rstd = vpool.tile([1, T], F32)

        def down_hband(hb_idx, h0, HB, psB):
            """Emit one down-proj h-band: stage/unpack + DR matmuls.
            Returns po tiles; caller emits the output scaling."""
            W = HB * 128
            po = [
                [
                    psB.tile([128, TC], F32, tag=f"po_{ht}_{t}", name=f"po{hb_idx}_{ht}_{t}")
                    for t in range(NT)
                ]
                for ht in range(HB)
            ]
            for C in range(n_cgrp):
                tail = C >= NC_FULL
                ks = range(2) if tail else range(4)
                k_last = 1 if tail else 3
                st = dstage.tile([128, 512], U8, tag="dst")
                nc.sync.dma_start(
                    st[:, :W], dpT[128 * C : 128 * (C + 1), h0 : h0 + W]
                )
                sh4 = dsh.tile([128, 4, 512], U8, tag="dsh")
                stw = st[:, :W].bitcast(U32)
                for k in ks:
                    nc.vector.tensor_scalar(
                        sh4[:, k, :W].bitcast(U32),
                        stw,
                        6 - 2 * k,
                        SWAR_MASK,
                        AOT.logical_shift_right,
                        AOT.bitwise_and,
                    )
                wd8 = wd.tile([128, 4, 512], FP8, tag="wd8")
                for k in ks:
                    if k < 2:
                        nc.scalar.activation(
                            wd8[:, k, :W], sh4[:, k, :W], ACTF.Copy, bias=-1.0
                        )
                    else:
                        nc.vector.tensor_scalar(
                            wd8[:, k, :W], sh4[:, k, :W], -1.0, None, AOT.add
                        )
                for k in ks:
                    rhs_tile = 4 * C + k
                    for ht in range(HB):
                        lhsT = bc2(wd8[:, k, 128 * ht : 128 * (ht + 1)])
                        for t in range(NT):
                            tsl = slice(TC * t, TC * (t + 1))
                            nc.tensor.matmul(
                                po[ht][t][:, :],
                                lhsT,
                                h8[:, rhs_tile, :, tsl],
                                start=(C == 0 and k == 0),
                                stop=(C == n_cgrp - 1 and k == k_last),
                                perf_mode=DR,
                            )
                if hb_idx == 0 and C == 0:
                    # variance colsum right after the first C-group: PE
                    # arrives here ~2us after phase A, s_bf is ready by then
                    for t in range(NT):
                        tsl = slice(TC * t, TC * (t + 1))
                        nc.tensor.matmul(
                            pv[:, tsl], ones_col[:], s_bf[:, tsl],
                            start=True, stop=True,
                        )
                    nc.scalar.activation(
                        var_sb[:], pv[:], ACTF.Copy, bias=RMS_EPS, scale=a_t[:]
                    )
                    nc.vector.reciprocal_approx_fast(rv[:], var_sb[:])
                    nc.scalar.activation(rstd[:], rv[:], ACTF.Sqrt, scale=d_t[:])
            return po

        def emit_out(po, h0, HB):
            for ht in range(HB):
                for t in range(NT):
                    tsl = slice(TC * t, TC * (t + 1))
                    ot = outp.tile([128, TC], BF16, tag="ot")
                    nc.vector.tensor_mul(ot[:], po[ht][t][:], rstd_bc[:, tsl])
                    nc.sync.dma_start(
                        outT[h0 + 128 * ht : h0 + 128 * (ht + 1), tsl], ot[:]
                    )

        with (
            tc.tile_pool(name="dstage", bufs=3) as dstage,
            tc.tile_pool(name="dsh", bufs=2) as dsh,
            tc.tile_pool(name="wd", bufs=2) as wd,
            tc.tile_pool(name="outp", bufs=3) as outp,
        ):
            # hband 0 (HB=2, 4 PSUM banks) alongside pv + ps_bc (4 banks)
            with tc.tile_pool(name="psV", bufs=1, space="PSUM") as psV:
                pv = psV.tile([1, T], F32, name="pv")
                with tc.tile_pool(name="psB0", bufs=1, space="PSUM") as psB0:
                    HB0 = HBANDS[0]
                    po0 = down_hband(0, 0, HB0, psB0)
                    # broadcast rstd to 128 partitions via K=1 matmul
                    ps_bc = psV.tile([128, T], F32, name="ps_bc")
                    for t in range(NT):
                        tsl = slice(TC * t, TC * (t + 1))
                        nc.tensor.matmul(
                            ps_bc[:, tsl], ones_row[:], rstd[:, tsl],
                            start=True, stop=True,
                        )
                    nc.scalar.activation(rstd_bc[:], ps_bc[:], ACTF.Copy)
                    emit_out(po0, 0, HB0)
            with tc.tile_pool(name="psB", bufs=1, space="PSUM") as psB:
                h0 = HBANDS[0] * 128
                for hb in range(1, len(HBANDS)):
                    HB = HBANDS[hb]
                    po = down_hband(hb, h0, HB, psB)
                    emit_out(po, h0, HB)
                    h0 += HB * 128

    nc.compile()
    return nc


# ------------------------------------------------------------- host-side prep
def unpack_host(packed, K):
    """[M, K//4] u8 -> [M, K] int8 ternary-ish {-1,0,1,2} (SIMD block order)."""
    M = packed.shape[0]
    b = packed.astype(np.int16).reshape(M, K // 128, 32)
    w = np.stack([(b >> 6) & 3, (b >> 4) & 3, (b >> 2) & 3, b & 3], axis=2)
    return (w.reshape(M, K) - 1).astype(np.int8)


def prep_inputs(x, gate_packed, gate_scale, up_packed, up_scale, down_packed,
                down_scale, norm_w, n_cores):
    B, S, H = x.shape
    I = norm_w.shape[0]
    T_full = B * S
    T = T_full // n_cores
    piH = perm_H(H)
    piI = perm_I(I)

    gs_v = float(np.asarray(gate_scale).reshape(-1)[0])
    us_v = float(np.asarray(up_scale).reshape(-1)[0])
    ds_v = float(np.asarray(down_scale).reshape(-1)[0])

    xf = np.ascontiguousarray(x.reshape(T_full, H), dtype=np.float32)
    # power-of-2 scale so x*s_x fits e4m3 comfortably
    s_x = 2.0 ** np.floor(np.log2(224.0 / float(np.abs(xf).max())))
    xs = xf * np.float32(s_x)
    x_hi = xs.astype(E4M3)
    x_lo = (xs - x_hi.astype(np.float32)).astype(E4M3)

    # estimate hidden absmax from a token sample to pick the fp8 range scale
    Wg_t = unpack_host(np.asarray(gate_packed, np.uint8), H)
    Wu_t = unpack_host(np.asarray(up_packed, np.uint8), H)
    idx = np.linspace(0, T_full - 1, 24).astype(np.int64)
    xr_s = x_hi[idx].astype(np.float32) + x_lo[idx].astype(np.float32)
    G_s = xr_s @ Wg_t.T.astype(np.float32)
    U_s = xr_s @ Wu_t.T.astype(np.float32)
    h_s = np.square(np.maximum(G_s, 0.0)) * U_s
    M_est = float(np.abs(h_s * norm_w[None, :].astype(np.float32)).max())
    k_h = 2.0 ** np.floor(np.log2(448.0 / (32.0 * M_est)))

    c_h = s_x**3 / (gs_v * gs_v * us_v)
    q = c_h * k_h
    A = 1.0 / (I * c_h * c_h)
    D = (ds_v / q) ** 2

    gpTp = np.ascontiguousarray(gate_packed[piI].T)  # [H/4, I]
    upTp = np.ascontiguousarray(up_packed[piI].T)
    dpT = np.ascontiguousarray(down_packed.T)  # [I/4, H]
    if I % 512:
        # re-pack the 64 ragged tail rows into one full 128-row block whose
        # shift-0/1 fields hold the tail i-values in h8 tile-52/53 partition
        # order (shift-2/3 fields encode weight 0)
        n_full_rows = (dpT.shape[0] // 128) * 128
        n_full_i = (I // 512) * 512
        Wd_t = unpack_host(np.asarray(down_packed, np.uint8), I)  # [H, I]
        p = np.arange(128)
        i52 = n_full_i + 128 * ((p % 64) // 32) + 32 * (p // 64) + (p % 32)
        w0 = (Wd_t[:, i52].astype(np.int16) + 1).astype(np.uint8)
        w1 = (Wd_t[:, i52 + 64].astype(np.int16) + 1).astype(np.uint8)
        pad = (w0 << 6) | (w1 << 4) | (1 << 2) | 1  # [H, 128]
        dpT = np.ascontiguousarray(
            np.concatenate([dpT[:n_full_rows], pad.T], axis=0)
        )
    nwk = np.ascontiguousarray(norm_w[piI]).astype(np.float32) * np.float32(k_h)

    in_maps = []
    for cid in range(n_cores):
        tsl = slice(cid * T, (cid + 1) * T)
        x8p = np.ascontiguousarray(
            np.stack([x_hi[tsl].T[piH], x_lo[tsl].T[piH]], axis=1)
        )  # [H, 2, T]
        in_maps.append(
            {
                "x8p": x8p,
                "gpTp": gpTp,
                "upTp": upTp,
                "dpT": dpT,
                "nwk": nwk,
                "a_in": np.asarray([A], np.float32),
                "d_in": np.asarray([D], np.float32),
            }
        )
    return in_maps


def assemble_output(results, B, S, H):
    outs = [np.asarray(r["outT"]).astype(np.float32).T for r in results]
    return np.ascontiguousarray(np.concatenate(outs, axis=0).reshape(B, S, H))


# ---------------------------------------------------------------- entry point
_CACHED = {}


def _get_program():
    if "nc" not in _CACHED:
        T = FULL_B * FULL_S // N_CORES
        _CACHED["nc"] = build_program(T, FULL_H, FULL_I)
    return _CACHED["nc"]


def kernel(x, gate_packed, gate_scale, up_packed, up_scale, down_packed,
           down_scale, norm_w, _trace=False):
    from concourse.bass_utils import run_bass_kernel_spmd

    x = np.asarray(x, np.float32)
    gate_packed = np.asarray(gate_packed, np.uint8)
    up_packed = np.asarray(up_packed, np.uint8)
    down_packed = np.asarray(down_packed, np.uint8)
    norm_w = np.asarray(norm_w, np.float32)

    B, S, H = x.shape
    in_maps = prep_inputs(
        x, gate_packed, gate_scale, up_packed, up_scale, down_packed,
        down_scale, norm_w, N_CORES,
    )
    nc = _get_program()
    res = run_bass_kernel_spmd(nc, in_maps, list(range(N_CORES)), trace=_trace)
    out = assemble_output(res.results, B, S, H)
    if _trace:
        kernel.last_results = res
    return out


# revision 9
# speedup vs baseline: 1.0756x; 1.0468x over previous
"""BitNet MLP (nn_BitNetMLP_19421842112750) — TRN2 Bass kernel, 8-core
data-parallel over tokens, fp8 DoubleRow matmuls with exact hi/lo splitting.

Per core (T=1024 tokens of the 8192 total):
  G = x @ Wg_tern.T ; U = x @ Wu_tern.T
  h = relu(G)^2 * U
  var = (sum_i h^2)*A + eps ; rstd = sqrt(D / var)
  out = ((h*nw*k) @ Wd_tern.T) * rstd

Matmuls run as fp8e4 DoubleRow with zero weight-quantization error (ternary
{-1,0,1,2} is exact in e4m3) and ~9-bit effective activation mantissa: each
activation v is split into (hi, lo) = (e4m3(v), e4m3(v - hi)); the
stationary weight is broadcast (stride-0) along the DoubleRow pair dim so
each cell computes w*hi + w*lo = w*v. This matches bf16 PE throughput (the
pair doubles both MACs and MAC-rate) but halves SBUF/DMA traffic for x and
the hidden layer. x is split on the host; hidden on DVE.

Weight unpack (2-bit -> fp8e4) runs on device: DVE SWAR shift/mask then a
convert(+-1) split between ACT and DVE. Permutations pi_H / pi_I absorb the
unpack partition scramble. The ragged 64-row tail of down_packed is
re-packed host-side into one full 128-row block (shift-0/1 fields).

Device layouts:
  x8p  [H, 2, T]  fp8  rows: h = pi_H(r), pair dim = (hi, lo)
  gpTp [H/4, I]   u8   cols: i = pi_I(c)   (same for upTp)
  dpT  [IB_PAD, H] u8  natural + re-packed tail block
  nwk  [I]        f32  nwk[r] = norm_w[pi_I(r)] * k_h
  outT [H, T]     bf16 natural h rows (host transposes back)
"""

import sys

sys.path.insert(0, "/opt/trn_rl_repo")
from contextlib import ExitStack

import numpy as np
import ml_dtypes

import concourse.bass as bass
import concourse.tile as tile
from concourse import bacc, mybir

F32 = mybir.dt.float32
BF16 = mybir.dt.bfloat16
U8 = mybir.dt.uint8
U32 = mybir.dt.uint32
FP8 = mybir.dt.float8e4
SWAR_MASK = 0x03030303
AOT = mybir.AluOpType
ACTF = mybir.ActivationFunctionType
DR = mybir.MatmulPerfMode.DoubleRow
E4M3 = ml_dtypes.float8_e4m3fn
RMS_EPS = 1e-6

N_CORES = 8
FULL_B, FULL_S, FULL_H, FULL_I = 4, 2048, 2560, 6912


# ---------------------------------------------------------------- permutations
def perm_H(n):
    """SBUF row r -> original h index. Groups of 512 (4 chunks x 128)."""
    assert n % 512 == 0
    r = np.arange(n)
    c, p = r // 128, r % 128
    return 512 * (c // 4) + 128 * (p // 32) + 32 * (c % 4) + (p % 32)


def perm_I(n):
    """hidden SBUF row r -> original i index. Full 512-groups, then a
    256-tail (two 128-tiles, each split into 64-partition halves)."""
    r = np.arange(n)
    c, p = r // 128, r % 128
    out = 512 * (c // 4) + 128 * (p // 32) + 32 * (c % 4) + (p % 32)
    n_full = (n // 512) * 512
    if n_full != n:
        assert n - n_full == 256, "tail must be exactly 256"
        off = r[n_full:] - n_full
        tile_off, p2 = off // 128, off % 128
        s, q, j = p2 // 64, (p2 % 64) // 32, p2 % 32
        k = 2 * tile_off + s
        out[n_full:] = n_full + 128 * q + 32 * k + j
    return out


# ---------------------------------------------------------------- the program
def build_program(T, H, I):
    """Build the single-core Bass program (SPMD-identical across cores)."""
    NH = H // 128          # x chunks / gate-up contraction chunks (20)
    NGH = H // 512         # packed row groups per gate/up band (5)
    NI = I // 128          # hidden i-tiles (54)
    NB = NI                # gate/up bands (W_I = 128)
    TC = 512
    NT = T // TC           # 2
    IB = I // 4            # down packed rows (1728)
    NC_FULL = IB // 128    # 13
    C_TAIL = IB % 128      # 64
    IB_PAD = (NC_FULL + 1) * 128 if C_TAIL else IB
    n_cgrp = NC_FULL + (1 if C_TAIL else 0)
    assert (NH - 4) % 4 == 0
    HBANDS = [3] + [4] * ((NH - 4) // 4) + [1]
    assert sum(HBANDS) == NH

    nc = bacc.Bacc("TRN2", target_bir_lowering=False, debug=False)

    x8p = nc.dram_tensor("x8p", [H, 2, T], FP8, kind="ExternalInput").ap()
    gpTp = nc.dram_tensor("gpTp", [H // 4, I], U8, kind="ExternalInput").ap()
    upTp = nc.dram_tensor("upTp", [H // 4, I], U8, kind="ExternalInput").ap()
    dpT = nc.dram_tensor("dpT", [IB_PAD, H], U8, kind="ExternalInput").ap()
    nwk = nc.dram_tensor("nwk", [I], F32, kind="ExternalInput").ap()
    a_in = nc.dram_tensor("a_in", [1], F32, kind="ExternalInput").ap()
    d_in = nc.dram_tensor("d_in", [1], F32, kind="ExternalInput").ap()
    outT = nc.dram_tensor("outT", [H, T], BF16, kind="ExternalOutput").ap()

    def bc2(w):  # stationary [K, M] -> [K, 2, M] stride-0 DoubleRow pair
        return w.unsqueeze(1).broadcast_to([w.shape[0], 2, w.shape[1]])

    with tile.TileContext(nc) as tc, ExitStack() as top:
        const = top.enter_context(tc.tile_pool(name="const", bufs=1))
        hpool = top.enter_context(tc.tile_pool(name="h8", bufs=1))
        h8 = hpool.tile([128, NI, 2, T], FP8)
        xpool = top.enter_context(tc.tile_pool(name="xT", bufs=1))
        wstage = top.enter_context(tc.tile_pool(name="wstage", bufs=2))
        ush = top.enter_context(tc.tile_pool(name="ush", bufs=2))
        wband = top.enter_context(tc.tile_pool(name="wband", bufs=2))
        dstage = top.enter_context(tc.tile_pool(name="dstage", bufs=3))
        dsh = top.enter_context(tc.tile_pool(name="dsh", bufs=2))
        wd = top.enter_context(tc.tile_pool(name="wd", bufs=2))
        outp = top.enter_context(tc.tile_pool(name="outp", bufs=3))

        # ---- gate/up band staging (DMA + SWAR unpack + fp8 convert)
        def stage_band(it):
            i0 = it * 128
            st = wstage.tile([128, 2, NGH * 128], U8, tag="st")
            for m, mat_ap in enumerate((gpTp, upTp)):
                nc.sync.dma_start(
                    st[:, m, :].rearrange("p (b w) -> p b w", b=NGH),
                    mat_ap.rearrange("(b p) w -> p b w", p=128)[:, :, i0 : i0 + 128],
                )
            sh = ush.tile([128, 4, 2, NGH * 128], U8, tag="sh")
            stw = st[:].bitcast(U32)
            for k in range(4):
                nc.vector.tensor_scalar(
                    sh[:, k, :, :].bitcast(U32), stw, 6 - 2 * k, SWAR_MASK,
                    AOT.logical_shift_right, AOT.bitwise_and,
                )
            wg8 = wband.tile([128, 4, NGH, 128], FP8, tag="wg")
            wu8 = wband.tile([128, 4, NGH, 128], FP8, tag="wu")
            for m, wt in ((0, wg8), (1, wu8)):
                src = sh[:, :, m, :].rearrange("p k (b w) -> p k b w", b=NGH)
                nc.scalar.activation(wt[:, 0:2], src[:, 0:2], ACTF.Copy, bias=-1.0)
                nc.vector.tensor_scalar(wt[:, 2:4], src[:, 2:4], -1.0, None, AOT.add)
            return wg8, wu8

        # band 0/1 weights first so the PE can start ~3us in, then x chunks
        staged_gu = {0: stage_band(0), 1: stage_band(1)}
        xts = []
        for c in range(NH):
            xt = xpool.tile([128, 2, T], FP8, name=f"x{c}")
            nc.sync.dma_start(xt[:], x8p[128 * c : 128 * (c + 1), :, :])
            xts.append(xt)

        # ---- constants
        a_t = const.tile([1, 1], F32)
        d_t = const.tile([1, 1], F32)
        nc.sync.dma_start(a_t[:], a_in[None, :])
        nc.sync.dma_start(d_t[:], d_in[None, :])
        nw_sb = const.tile([128, NI], F32)
        nc.sync.dma_start(nw_sb[:], nwk.rearrange("(o p) -> p o", p=128))
        s_acc = const.tile([128, T], F32)
        nc.vector.memset(s_acc[:], 0.0)
        s_bf = const.tile([128, T], BF16)
        ones_col = const.tile([128, 1], BF16)
        nc.vector.memset(ones_col[:], 1.0)
        ones_row = const.tile([1, 128], F32)
        nc.vector.memset(ones_row[:], 1.0)
        rstd_bc = const.tile([128, T], F32)

        # ---- down-proj C-group staging (DMA + unpack + convert)
        def stage_down(C, h0, W):
            tail = C >= NC_FULL
            st = dstage.tile([128, 512], U8, tag="dst")
            nc.sync.dma_start(st[:, :W], dpT[128 * C : 128 * (C + 1), h0 : h0 + W])
            sh4 = dsh.tile([128, 4, 512], U8, tag="dsh")
            stw = st[:, :W].bitcast(U32)
            for k in range(2) if tail else range(4):
                nc.vector.tensor_scalar(
                    sh4[:, k, :W].bitcast(U32), stw, 6 - 2 * k, SWAR_MASK,
                    AOT.logical_shift_right, AOT.bitwise_and,
                )
            wd8 = wd.tile([128, 4, 512], FP8, tag="wd8")
            nc.scalar.activation(
                wd8[:, 0:2, :W], sh4[:, 0:2, :W], ACTF.Copy, bias=-1.0
            )
            if not tail:
                nc.vector.tensor_scalar(
                    wd8[:, 2:4, :W], sh4[:, 2:4, :W], -1.0, None, AOT.add
                )
            return wd8

        # ================= phase A: gate/up DoubleRow matmuls ===============
        with (
            tc.tile_pool(name="psA", bufs=2, space="PSUM") as psA,
            tc.tile_pool(name="ract", bufs=2) as ract,
        ):
            for it in range(NB):
                wg8, wu8 = staged_gu.pop(it, None) or stage_band(it)
                pg = psA.tile([128, T], F32, tag="pg")
                pu = psA.tile([128, T], F32, tag="pu")
                for c in range(NH):
                    Bq, k = divmod(c, 4)
                    for ps_t, wt in ((pg, wg8), (pu, wu8)):
                        lhsT = bc2(wt[:, k, Bq, :])
                        for t in range(NT):
                            tsl = slice(TC * t, TC * (t + 1))
                            nc.tensor.matmul(
                                ps_t[:, tsl], lhsT, xts[c][:, :, tsl],
                                start=(c == 0), stop=(c == NH - 1),
                                perf_mode=DR,
                            )
                # r-stage: h = relu(G)^2 * U ; s_acc += h^2 ; h8 = split(h*nw)
                q = ract.tile([128, T], BF16, tag="q")
                h2t = ract.tile([128, T], F32, tag="h2t")
                nc.scalar.activation(q[:], pg[:], ACTF.Relu)
                nc.vector.tensor_mul(q[:], q[:], q[:])
                nc.vector.tensor_mul(q[:], q[:], pu[:])
                nc.vector.tensor_mul(h2t[:], q[:], q[:])
                nc.vector.tensor_tensor(s_acc[:], s_acc[:], h2t[:], AOT.add)
                nwc = nw_sb[:, it : it + 1]
                nc.vector.tensor_scalar(h8[:, it, 0, :], q[:], nwc, None, AOT.mult)
                nc.vector.scalar_tensor_tensor(
                    h8[:, it, 1, :], q[:], nwc, h8[:, it, 0, :],
                    AOT.mult, AOT.subtract,
                )
                if it == NB - 1:
                    nc.vector.tensor_copy(s_bf[:], s_acc[:])
                if it == NB - 2:
                    # prefetch the first two down C-groups so phase B's
                    # matmuls can start right at phase A's end
                    staged_wd = [stage_down(0, 0, HBANDS[0] * 128),
                                 stage_down(1, 0, HBANDS[0] * 128)]

        # ================= phase B + variance finalization ==================
        vpool = top.enter_context(tc.tile_pool(name="vmisc", bufs=1))
        var_sb = vpool.tile([1, T], F32)
        rv = vpool.tile([1, T], F32)
        rstd = vpool.tile([1, T], F32)

        def down_hband(hb_idx, h0, HB, psB, pv=None, ps_bc=None):
            W = HB * 128
            po = [
                [
                    psB.tile([128, TC], F32, tag=f"po_{ht}_{t}",
                             name=f"po{hb_idx}_{ht}_{t}")
                    for t in range(NT)
                ]
                for ht in range(HB)
            ]
            for C in range(n_cgrp):
                tail = C >= NC_FULL
                ks = range(2) if tail else range(4)
                k_last = 1 if tail else 3
                if hb_idx == 0 and C < 2:
                    wd8 = staged_wd[C]
                else:
                    wd8 = stage_down(C, h0, W)
                for k in ks:
                    rhs_tile = 4 * C + k
                    for ht in range(HB):
                        lhsT = bc2(wd8[:, k, 128 * ht : 128 * (ht + 1)])
                        for t in range(NT):
                            tsl = slice(TC * t, TC * (t + 1))
                            nc.tensor.matmul(
                                po[ht][t][:, :], lhsT, h8[:, rhs_tile, :, tsl],
                                start=(C == 0 and k == 0),
                                stop=(C == n_cgrp - 1 and k == k_last),
                                perf_mode=DR,
                            )
                if hb_idx == 0 and C == 0:
                    # variance colsum + rstd chain: one PSUM bank, t-serial
                    for t in range(NT):
                        tsl = slice(TC * t, TC * (t + 1))
                        nc.tensor.matmul(pv[:], ones_col[:], s_bf[:, tsl],
                                         start=True, stop=True)
                        nc.scalar.activation(var_sb[:, tsl], pv[:], ACTF.Copy,
                                             bias=RMS_EPS, scale=a_t[:])
                    nc.vector.reciprocal_approx_fast(rv[:], var_sb[:])
                    nc.scalar.activation(rstd[:], rv[:], ACTF.Sqrt, scale=d_t[:])
                if hb_idx == 0 and C == 2:
                    # broadcast rstd to 128 partitions (chain done by now)
                    for t in range(NT):
                        tsl = slice(TC * t, TC * (t + 1))
                        nc.tensor.matmul(ps_bc[:], ones_row[:], rstd[:, tsl],
                                         start=True, stop=True)
                        nc.scalar.activation(rstd_bc[:, tsl], ps_bc[:], ACTF.Copy)
            return po

        def emit_out(po, h0, HB):
            for ht in range(HB):
                for t in range(NT):
                    tsl = slice(TC * t, TC * (t + 1))
                    ot = outp.tile([128, TC], BF16, tag="ot")
                    nc.vector.tensor_mul(ot[:], po[ht][t][:], rstd_bc[:, tsl])
                    nc.sync.dma_start(
                        outT[h0 + 128 * ht : h0 + 128 * (ht + 1), tsl], ot[:]
                    )

        with tc.tile_pool(name="psV", bufs=1, space="PSUM") as psV:
            pv = psV.tile([1, TC], F32, name="pv")
            ps_bc = psV.tile([128, TC], F32, name="ps_bc")
            with tc.tile_pool(name="psB0", bufs=1, space="PSUM") as psB0:
                po0 = down_hband(0, 0, HBANDS[0], psB0, pv=pv, ps_bc=ps_bc)
                emit_out(po0, 0, HBANDS[0])
        with tc.tile_pool(name="psB", bufs=1, space="PSUM") as psB:
            h0 = HBANDS[0] * 128
            for hb in range(1, len(HBANDS)):
                HB = HBANDS[hb]
                po = down_hband(hb, h0, HB, psB)
                emit_out(po, h0, HB)
                h0 += HB * 128

    nc.compile()
    return nc


# ------------------------------------------------------------- host-side prep
def unpack_host(packed, K):
    """[M, K//4] u8 -> [M, K] int8 {-1,0,1,2} (SIMD block order)."""
    M = packed.shape[0]
    b = packed.astype(np.int16).reshape(M, K // 128, 32)
    w = np.stack([(b >> 6) & 3, (b >> 4) & 3, (b >> 2) & 3, b & 3], axis=2)
    return (w.reshape(M, K) - 1).astype(np.int8)


def prep_inputs(x, gate_packed, gate_scale, up_packed, up_scale, down_packed,
                down_scale, norm_w, n_cores):
    B, S, H = x.shape
    I = norm_w.shape[0]
    T_full = B * S
    T = T_full // n_cores
    piH = perm_H(H)
    piI = perm_I(I)

    gs_v = float(np.asarray(gate_scale).reshape(-1)[0])
    us_v = float(np.asarray(up_scale).reshape(-1)[0])
    ds_v = float(np.asarray(down_scale).reshape(-1)[0])

    xf = np.ascontiguousarray(x.reshape(T_full, H), dtype=np.float32)
    # power-of-2 scale so x*s_x fits e4m3 comfortably
    s_x = 2.0 ** np.floor(np.log2(224.0 / float(np.abs(xf).max())))
    xs = xf * np.float32(s_x)
    x_hi = xs.astype(E4M3)
    x_lo = (xs - x_hi.astype(np.float32)).astype(E4M3)

    # estimate hidden absmax from a token sample to pick the fp8 range scale
    Wg_t = unpack_host(np.asarray(gate_packed, np.uint8), H)
    Wu_t = unpack_host(np.asarray(up_packed, np.uint8), H)
    idx = np.linspace(0, T_full - 1, 24).astype(np.int64)
    xr_s = x_hi[idx].astype(np.float32) + x_lo[idx].astype(np.float32)
    G_s = xr_s @ Wg_t.T.astype(np.float32)
    U_s = xr_s @ Wu_t.T.astype(np.float32)
    h_s = np.square(np.maximum(G_s, 0.0)) * U_s
    M_est = float(np.abs(h_s * norm_w[None, :].astype(np.float32)).max())
    k_h = 2.0 ** np.floor(np.log2(448.0 / (32.0 * M_est)))

    c_h = s_x**3 / (gs_v * gs_v * us_v)
    q = c_h * k_h
    A = 1.0 / (I * c_h * c_h)
    D = (ds_v / q) ** 2

    gpTp = np.ascontiguousarray(gate_packed[piI].T)  # [H/4, I]
    upTp = np.ascontiguousarray(up_packed[piI].T)
    dpT = np.ascontiguousarray(down_packed.T)  # [I/4, H]
    if I % 512:
        # re-pack the 64 ragged tail rows into one full 128-row block whose
        # shift-0/1 fields hold the tail i-values in h8 tile-52/53 partition
        # order (shift-2/3 fields encode weight 0)
        n_full_rows = (dpT.shape[0] // 128) * 128
        n_full_i = (I // 512) * 512
        Wd_t = unpack_host(np.asarray(down_packed, np.uint8), I)  # [H, I]
        p = np.arange(128)
        i52 = n_full_i + 128 * ((p % 64) // 32) + 32 * (p // 64) + (p % 32)
        w0 = (Wd_t[:, i52].astype(np.int16) + 1).astype(np.uint8)
        w1 = (Wd_t[:, i52 + 64].astype(np.int16) + 1).astype(np.uint8)
        pad = (w0 << 6) | (w1 << 4) | (1 << 2) | 1  # [H, 128]
        dpT = np.ascontiguousarray(
            np.concatenate([dpT[:n_full_rows], pad.T], axis=0)
        )
    nwk = np.ascontiguousarray(norm_w[piI]).astype(np.float32) * np.float32(k_h)

    in_maps = []
    for cid in range(n_cores):
        tsl = slice(cid * T, (cid + 1) * T)
        x8p = np.ascontiguousarray(
            np.stack([x_hi[tsl].T[piH], x_lo[tsl].T[piH]], axis=1)
        )  # [H, 2, T]
        in_maps.append(
            {
                "x8p": x8p,
                "gpTp": gpTp,
                "upTp": upTp,
                "dpT": dpT,
                "nwk": nwk,
                "a_in": np.asarray([A], np.float32),
                "d_in": np.asarray([D], np.float32),
            }
        )
    return in_maps


def assemble_output(results, B, S, H):
    outs = [np.asarray(r["outT"]).astype(np.float32).T for r in results]
    return np.ascontiguousarray(np.concatenate(outs, axis=0).reshape(B, S, H))


# ---------------------------------------------------------------- entry point
_CACHED = {}


def _get_program():
    if "nc" not in _CACHED:
        T = FULL_B * FULL_S // N_CORES
        _CACHED["nc"] = build_program(T, FULL_H, FULL_I)
    return _CACHED["nc"]


def kernel(x, gate_packed, gate_scale, up_packed, up_scale, down_packed,
           down_scale, norm_w, _trace=False):
    from concourse.bass_utils import run_bass_kernel_spmd

    x = np.asarray(x, np.float32)
    gate_packed = np.asarray(gate_packed, np.uint8)
    up_packed = np.asarray(up_packed, np.uint8)
    down_packed = np.asarray(down_packed, np.uint8)
    norm_w = np.asarray(norm_w, np.float32)

    B, S, H = x.shape
    in_maps = prep_inputs(
        x, gate_packed, gate_scale, up_packed, up_scale, down_packed,
        down_scale, norm_w, N_CORES,
    )
    nc = _get_program()
    res = run_bass_kernel_spmd(nc, in_maps, list(range(N_CORES)), trace=_trace)
    out = assemble_output(res.results, B, S, H)
    if _trace:
        kernel.last_results = res
    return out


# revision 19
# speedup vs baseline: 1.0787x; 1.0028x over previous
"""BitNet MLP (nn_BitNetMLP_19421842112750) — TRN2 Bass kernel, 8-core
data-parallel over tokens, fp8 DoubleRow matmuls with exact hi/lo splitting.

Per core (T=1024 tokens of the 8192 total):
  G = x @ Wg_tern.T ; U = x @ Wu_tern.T
  h = relu(G)^2 * U
  var = (sum_i h^2)*A + eps ; rstd = sqrt(D / var)
  out = ((h*nw*k) @ Wd_tern.T) * rstd

Matmuls run as fp8e4 DoubleRow with zero weight-quantization error (ternary
{-1,0,1,2} is exact in e4m3) and ~9-bit effective activation mantissa: each
activation v is split into (hi, lo) = (e4m3(v), e4m3(v - hi)); the
stationary weight is broadcast (stride-0) along the DoubleRow pair dim so
each cell computes w*hi + w*lo = w*v. This matches bf16 PE throughput (the
pair doubles both MACs and MAC-rate) but halves SBUF/DMA traffic for x and
the hidden layer. x is split on the host; hidden on DVE.

Weight unpack (2-bit -> fp8e4) runs on device: DVE SWAR shift/mask then a
convert(+-1) split between ACT and DVE. Permutations pi_H / pi_I absorb the
unpack partition scramble. The ragged 64-row tail of down_packed is
re-packed host-side into one full 128-row block (shift-0/1 fields).

Device layouts:
  x8p  [H, 2, T]  fp8  rows: h = pi_H(r), pair dim = (hi, lo)
  gpTp [H/4, I]   u8   cols: i = pi_I(c)   (same for upTp)
  dpT  [IB_PAD, H] u8  natural + re-packed tail block
  nwk  [I]        f32  nwk[r] = norm_w[pi_I(r)] * k_h
  outT [H, T]     bf16 natural h rows (host transposes back)
"""

import sys

sys.path.insert(0, "/opt/trn_rl_repo")
from contextlib import ExitStack

import numpy as np
import ml_dtypes

import concourse.bass as bass
import concourse.tile as tile
from concourse import bacc, mybir

F32 = mybir.dt.float32
BF16 = mybir.dt.bfloat16
U8 = mybir.dt.uint8
U32 = mybir.dt.uint32
FP8 = mybir.dt.float8e4
SWAR_MASK = 0x03030303
AOT = mybir.AluOpType
ACTF = mybir.ActivationFunctionType
DR = mybir.MatmulPerfMode.DoubleRow
E4M3 = ml_dtypes.float8_e4m3fn
RMS_EPS = 1e-6

N_CORES = 8
FULL_B, FULL_S, FULL_H, FULL_I = 4, 2048, 2560, 6912


# ---------------------------------------------------------------- permutations
def perm_H(n):
    """SBUF row r -> original h index. Groups of 512 (4 chunks x 128)."""
    assert n % 512 == 0
    r = np.arange(n)
    c, p = r // 128, r % 128
    return 512 * (c // 4) + 128 * (p // 32) + 32 * (c % 4) + (p % 32)


def perm_I(n):
    """hidden SBUF row r -> original i index. Full 512-groups, then a
    256-tail (two 128-tiles, each split into 64-partition halves)."""
    r = np.arange(n)
    c, p = r // 128, r % 128
    out = 512 * (c // 4) + 128 * (p // 32) + 32 * (c % 4) + (p % 32)
    n_full = (n // 512) * 512
    if n_full != n:
        assert n - n_full == 256, "tail must be exactly 256"
        off = r[n_full:] - n_full
        tile_off, p2 = off // 128, off % 128
        s, q, j = p2 // 64, (p2 % 64) // 32, p2 % 32
        k = 2 * tile_off + s
        out[n_full:] = n_full + 128 * q + 32 * k + j
    return out


# ---------------------------------------------------------------- the program
def build_program(T, H, I):
    """Build the single-core Bass program (SPMD-identical across cores)."""
    NH = H // 128          # x chunks / gate-up contraction chunks (20)
    NGH = H // 512         # packed row groups per gate/up band (5)
    NI = I // 128          # hidden i-tiles (54)
    NB = NI                # gate/up bands (W_I = 128)
    TC = 512
    NT = T // TC           # 2
    IB = I // 4            # down packed rows (1728)
    NC_FULL = IB // 128    # 13
    C_TAIL = IB % 128      # 64
    IB_PAD = (NC_FULL + 1) * 128 if C_TAIL else IB
    n_cgrp = NC_FULL + (1 if C_TAIL else 0)
    assert (NH - 4) % 4 == 0
    HBANDS = [3] + [4] * ((NH - 4) // 4) + [1]
    assert sum(HBANDS) == NH

    nc = bacc.Bacc("TRN2", target_bir_lowering=False, debug=False)

    x8p = nc.dram_tensor("x8p", [H, 2, T], FP8, kind="ExternalInput").ap()
    gub = nc.dram_tensor(
        "gub", [128, NB, 2, NGH * 128], U8, kind="ExternalInput"
    ).ap()
    dpT = nc.dram_tensor("dpT", [IB_PAD, H], U8, kind="ExternalInput").ap()
    nwk = nc.dram_tensor("nwk", [I], F32, kind="ExternalInput").ap()
    a_in = nc.dram_tensor("a_in", [1], F32, kind="ExternalInput").ap()
    d_in = nc.dram_tensor("d_in", [1], F32, kind="ExternalInput").ap()
    outT = nc.dram_tensor("outT", [H, T], BF16, kind="ExternalOutput").ap()

    def bc2(w):  # stationary [K, M] -> [K, 2, M] stride-0 DoubleRow pair
        return w.unsqueeze(1).broadcast_to([w.shape[0], 2, w.shape[1]])

    with tile.TileContext(nc) as tc, ExitStack() as top:
        const = top.enter_context(tc.tile_pool(name="const", bufs=1))
        hpool = top.enter_context(tc.tile_pool(name="h8", bufs=1))
        h8 = hpool.tile([128, NI, 2, T], FP8)
        xpool = top.enter_context(tc.tile_pool(name="xT", bufs=1))
        wstage = top.enter_context(tc.tile_pool(name="wstage", bufs=2))
        ush = top.enter_context(tc.tile_pool(name="ush", bufs=1))
        wband = top.enter_context(tc.tile_pool(name="wband", bufs=2))
        dstage = top.enter_context(tc.tile_pool(name="dstage", bufs=5))
        dsh = top.enter_context(tc.tile_pool(name="dsh", bufs=3))
        wd = top.enter_context(tc.tile_pool(name="wd", bufs=3))
        outp = top.enter_context(tc.tile_pool(name="outp", bufs=3))

        # ---- gate/up band staging (DMA + SWAR unpack + fp8 convert)
        def stage_band(it):
            st = wstage.tile([128, 2, NGH * 128], U8, tag="st")
            nc.sync.dma_start(st[:], gub[:, it])
            sh = ush.tile([128, 4, 2, NGH * 128], U8, tag="sh")
            stw = st[:].bitcast(U32)
            for k in range(4):
                nc.vector.tensor_scalar(
                    sh[:, k, :, :].bitcast(U32), stw, 6 - 2 * k, SWAR_MASK,
                    AOT.logical_shift_right, AOT.bitwise_and,
                )
            wg8 = wband.tile([128, 4, NGH, 128], FP8, tag="wg")
            wu8 = wband.tile([128, 4, NGH, 128], FP8, tag="wu")
            for m, wt in ((0, wg8), (1, wu8)):
                src = sh[:, :, m, :].rearrange("p k (b w) -> p k b w", b=NGH)
                nc.scalar.activation(wt[:], src, ACTF.Copy, bias=-1.0)
            return wg8, wu8

        # band 0/1 weights first so the PE can start ~3us in, then x chunks
        staged_gu = {0: stage_band(0), 1: stage_band(1)}
        xts = []
        for c in range(NH):
            xt = xpool.tile([128, 2, T], FP8, name=f"x{c}")
            nc.sync.dma_start(xt[:], x8p[128 * c : 128 * (c + 1), :, :])
            xts.append(xt)

        # ---- constants
        a_t = const.tile([1, 1], F32)
        d_t = const.tile([1, 1], F32)
        nc.sync.dma_start(a_t[:], a_in[None, :])
        nc.sync.dma_start(d_t[:], d_in[None, :])
        nw_sb = const.tile([128, NI], F32)
        nc.sync.dma_start(nw_sb[:], nwk.rearrange("(o p) -> p o", p=128))
        s_acc = const.tile([128, T], F32)
        nc.vector.memset(s_acc[:], 0.0)
        s_bf = const.tile([128, T], BF16)
        ones_col = const.tile([128, 1], BF16)
        nc.vector.memset(ones_col[:], 1.0)
        ones_row = const.tile([1, 128], F32)
        nc.vector.memset(ones_row[:], 1.0)
        rstd_bc = const.tile([128, T], F32)

        # ---- down-proj C-group staging (DMA / unpack + convert)
        def stage_down_dma(C, h0, W):
            st = dstage.tile([128, 512], U8, tag="dst")
            nc.sync.dma_start(st[:, :W], dpT[128 * C : 128 * (C + 1), h0 : h0 + W])
            return st

        def unpack_down(st, C, W):
            tail = C >= NC_FULL
            sh4 = dsh.tile([128, 4, 512], U8, tag="dsh")
            stw = st[:, :W].bitcast(U32)
            for k in range(2) if tail else range(4):
                nc.vector.tensor_scalar(
                    sh4[:, k, :W].bitcast(U32), stw, 6 - 2 * k, SWAR_MASK,
                    AOT.logical_shift_right, AOT.bitwise_and,
                )
            wd8 = wd.tile([128, 4, 512], FP8, tag="wd8")
            kk = 2 if tail else 4
            nc.scalar.activation(
                wd8[:, 0:kk, :W], sh4[:, 0:kk, :W], ACTF.Copy, bias=-1.0
            )
            return wd8

        def stage_down(C, h0, W):
            return unpack_down(stage_down_dma(C, h0, W), C, W)

        # ================= phase A: gate/up DoubleRow matmuls ===============
        with (
            tc.tile_pool(name="psA", bufs=2, space="PSUM") as psA,
            tc.tile_pool(name="ract", bufs=2) as ract,
        ):
            for it in range(NB):
                wg8, wu8 = staged_gu.pop(it, None) or stage_band(it)
                pg = psA.tile([128, T], F32, tag="pg")
                pu = psA.tile([128, T], F32, tag="pu")
                for c in range(NH):
                    Bq, k = divmod(c, 4)
                    for ps_t, wt in ((pg, wg8), (pu, wu8)):
                        lhsT = bc2(wt[:, k, Bq, :])
                        for t in range(NT):
                            tsl = slice(TC * t, TC * (t + 1))
                            nc.tensor.matmul(
                                ps_t[:, tsl], lhsT, xts[c][:, :, tsl],
                                start=(c == 0), stop=(c == NH - 1),
                                perf_mode=DR,
                            )
                # r-stage: h = relu(G)^2 * U ; s_acc += h^2 ; h8 = split(h*nw)
                q = ract.tile([128, T], BF16, tag="q")
                h2t = ract.tile([128, T], F32, tag="h2t")
                nc.scalar.activation(q[:], pg[:], ACTF.Relu)
                nc.vector.tensor_mul(q[:], q[:], q[:])
                nc.vector.tensor_mul(q[:], q[:], pu[:])
                nc.vector.tensor_mul(h2t[:], q[:], q[:])
                nc.vector.tensor_tensor(s_acc[:], s_acc[:], h2t[:], AOT.add)
                nwc = nw_sb[:, it : it + 1]
                nc.vector.tensor_scalar(h8[:, it, 0, :], q[:], nwc, None, AOT.mult)
                nc.vector.scalar_tensor_tensor(
                    h8[:, it, 1, :], q[:], nwc, h8[:, it, 0, :],
                    AOT.mult, AOT.subtract,
                )
                if it == NB - 1:
                    nc.vector.tensor_copy(s_bf[:], s_acc[:])
                if it == NB - 2:
                    # prefetch the first two down C-groups so phase B's
                    # matmuls can start right at phase A's end
                    staged_wd = [stage_down(0, 0, HBANDS[0] * 128),
                                 stage_down(1, 0, HBANDS[0] * 128)]

        # ================= phase B + variance finalization ==================
        vpool = top.enter_context(tc.tile_pool(name="vmisc", bufs=1))
        var_sb = vpool.tile([1, T], F32)
        rv = vpool.tile([1, T], F32)
        rstd = vpool.tile([1, T], F32)

        st_pre = {}

        def down_hband(hb_idx, h0, HB, psB, next_spec=None, pv=None, ps_bc=None):
            W = HB * 128
            po = [
                [
                    psB.tile([128, TC], F32, tag=f"po_{ht}_{t}",
                             name=f"po{hb_idx}_{ht}_{t}")
                    for t in range(NT)
                ]
                for ht in range(HB)
            ]
            for C in range(n_cgrp):
                tail = C >= NC_FULL
                ks = range(2) if tail else range(4)
                k_last = 1 if tail else 3
                if hb_idx == 0 and C < 2:
                    wd8 = staged_wd[C]
                else:
                    st = st_pre.pop((hb_idx, C), None)
                    if st is None:
                        st = stage_down_dma(C, h0, W)
                    wd8 = unpack_down(st, C, W)
                for k in ks:
                    rhs_tile = 4 * C + k
                    for ht in range(HB):
                        lhsT = bc2(wd8[:, k, 128 * ht : 128 * (ht + 1)])
                        for t in range(NT):
                            tsl = slice(TC * t, TC * (t + 1))
                            nc.tensor.matmul(
                                po[ht][t][:, :], lhsT, h8[:, rhs_tile, :, tsl],
                                start=(C == 0 and k == 0),
                                stop=(C == n_cgrp - 1 and k == k_last),
                                perf_mode=DR,
                            )
                if hb_idx == 0 and C == 1:
                    # variance colsum + rstd chain: one PSUM bank, t-serial
                    for t in range(NT):
                        tsl = slice(TC * t, TC * (t + 1))
                        nc.tensor.matmul(pv[:], ones_col[:], s_bf[:, tsl],
                                         start=True, stop=True)
                        nc.scalar.activation(var_sb[:, tsl], pv[:], ACTF.Copy,
                                             bias=RMS_EPS, scale=a_t[:])
                    nc.vector.reciprocal_approx_fast(rv[:], var_sb[:])
                    nc.scalar.activation(rstd[:], rv[:], ACTF.Sqrt, scale=d_t[:])
                if hb_idx == 0 and C == min(3, n_cgrp - 1):
                    # broadcast rstd to 128 partitions (chain done by now)
                    for t in range(NT):
                        tsl = slice(TC * t, TC * (t + 1))
                        nc.tensor.matmul(ps_bc[:], ones_row[:], rstd[:, tsl],
                                         start=True, stop=True)
                        nc.scalar.activation(rstd_bc[:, tsl], ps_bc[:], ACTF.Copy)
                if next_spec is not None and C == n_cgrp - 3:
                    nh0, nHB = next_spec
                    for Cp in range(2):
                        st_pre[(hb_idx + 1, Cp)] = stage_down_dma(
                            Cp, nh0, nHB * 128
                        )
            return po

        def emit_out(po, h0, HB):
            for ht in range(HB):
                for t in range(NT):
                    tsl = slice(TC * t, TC * (t + 1))
                    ot = outp.tile([128, TC], BF16, tag="ot")
                    nc.vector.tensor_mul(ot[:], po[ht][t][:], rstd_bc[:, tsl])
                    nc.sync.dma_start(
                        outT[h0 + 128 * ht : h0 + 128 * (ht + 1), tsl], ot[:]
                    )

        with tc.tile_pool(name="psV", bufs=1, space="PSUM") as psV:
            pv = psV.tile([1, TC], F32, name="pv")
            ps_bc = psV.tile([128, TC], F32, name="ps_bc")
            with tc.tile_pool(name="psB0", bufs=1, space="PSUM") as psB0:
                po0 = down_hband(0, 0, HBANDS[0], psB0,
                                 next_spec=(HBANDS[0] * 128, HBANDS[1]),
                                 pv=pv, ps_bc=ps_bc)
                emit_out(po0, 0, HBANDS[0])
        with tc.tile_pool(name="psB", bufs=1, space="PSUM") as psB:
            h0 = HBANDS[0] * 128
            for hb in range(1, len(HBANDS)):
                HB = HBANDS[hb]
                nspec = None
                if hb + 1 < len(HBANDS):
                    nspec = (h0 + HB * 128, HBANDS[hb + 1])
                po = down_hband(hb, h0, HB, psB, next_spec=nspec)
                emit_out(po, h0, HB)
                h0 += HB * 128

    nc.compile()
    return nc


# ------------------------------------------------------------- host-side prep
def unpack_host(packed, K):
    """[M, K//4] u8 -> [M, K] int8 {-1,0,1,2} (SIMD block order)."""
    M = packed.shape[0]
    b = packed.astype(np.int16).reshape(M, K // 128, 32)
    w = np.stack([(b >> 6) & 3, (b >> 4) & 3, (b >> 2) & 3, b & 3], axis=2)
    return (w.reshape(M, K) - 1).astype(np.int8)


def prep_inputs(x, gate_packed, gate_scale, up_packed, up_scale, down_packed,
                down_scale, norm_w, n_cores):
    B, S, H = x.shape
    I = norm_w.shape[0]
    T_full = B * S
    T = T_full // n_cores
    piH = perm_H(H)
    piI = perm_I(I)

    gs_v = float(np.asarray(gate_scale).reshape(-1)[0])
    us_v = float(np.asarray(up_scale).reshape(-1)[0])
    ds_v = float(np.asarray(down_scale).reshape(-1)[0])

    xf = np.ascontiguousarray(x.reshape(T_full, H), dtype=np.float32)
    # power-of-2 scale so x*s_x fits e4m3 comfortably
    s_x = 2.0 ** np.floor(np.log2(224.0 / float(np.abs(xf).max())))
    xs = xf * np.float32(s_x)
    x_hi = xs.astype(E4M3)
    x_lo = (xs - x_hi.astype(np.float32)).astype(E4M3)

    # estimate hidden absmax from a token sample to pick the fp8 range scale
    Wg_t = unpack_host(np.asarray(gate_packed, np.uint8), H)
    Wu_t = unpack_host(np.asarray(up_packed, np.uint8), H)
    idx = np.linspace(0, T_full - 1, 24).astype(np.int64)
    xr_s = x_hi[idx].astype(np.float32) + x_lo[idx].astype(np.float32)
    G_s = xr_s @ Wg_t.T.astype(np.float32)
    U_s = xr_s @ Wu_t.T.astype(np.float32)
    h_s = np.square(np.maximum(G_s, 0.0)) * U_s
    M_est = float(np.abs(h_s * norm_w[None, :].astype(np.float32)).max())
    k_h = 2.0 ** np.floor(np.log2(448.0 / (32.0 * M_est)))

    c_h = s_x**3 / (gs_v * gs_v * us_v)
    q = c_h * k_h
    A = 1.0 / (I * c_h * c_h)
    D = (ds_v / q) ** 2

    # band-contiguous gate/up packed bytes: gub[p, it, m, b*128+w] =
    # packed_m[piI[128*it + w], 128*b + p] -> one 1280B-per-partition DMA/band
    NB_, NGH_ = I // 128, H // 512
    gu = np.stack([gate_packed[piI], up_packed[piI]], axis=0)  # [2, I, H/4]
    gu = gu.reshape(2, NB_, 128, NGH_, 128)  # [m, it, w, b, p]
    gub = np.ascontiguousarray(
        gu.transpose(4, 1, 0, 3, 2).reshape(128, NB_, 2, NGH_ * 128)
    )
    dpT = np.ascontiguousarray(down_packed.T)  # [I/4, H]
    if I % 512:
        # re-pack the 64 ragged tail rows into one full 128-row block whose
        # shift-0/1 fields hold the tail i-values in h8 tile-52/53 partition
        # order (shift-2/3 fields encode weight 0)
        n_full_rows = (dpT.shape[0] // 128) * 128
        n_full_i = (I // 512) * 512
        Wd_t = unpack_host(np.asarray(down_packed, np.uint8), I)  # [H, I]
        p = np.arange(128)
        i52 = n_full_i + 128 * ((p % 64) // 32) + 32 * (p // 64) + (p % 32)
        w0 = (Wd_t[:, i52].astype(np.int16) + 1).astype(np.uint8)
        w1 = (Wd_t[:, i52 + 64].astype(np.int16) + 1).astype(np.uint8)
        pad = (w0 << 6) | (w1 << 4) | (1 << 2) | 1  # [H, 128]
        dpT = np.ascontiguousarray(
            np.concatenate([dpT[:n_full_rows], pad.T], axis=0)
        )
    nwk = np.ascontiguousarray(norm_w[piI]).astype(np.float32) * np.float32(k_h)

    in_maps = []
    for cid in range(n_cores):
        tsl = slice(cid * T, (cid + 1) * T)
        x8p = np.ascontiguousarray(
            np.stack([x_hi[tsl].T[piH], x_lo[tsl].T[piH]], axis=1)
        )  # [H, 2, T]
        in_maps.append(
            {
                "x8p": x8p,
                "gub": gub,
                "dpT": dpT,
                "nwk": nwk,
                "a_in": np.asarray([A], np.float32),
                "d_in": np.asarray([D], np.float32),
            }
        )
    return in_maps


def assemble_output(results, B, S, H):
    outs = [np.asarray(r["outT"]).astype(np.float32).T for r in results]
    return np.ascontiguousarray(np.concatenate(outs, axis=0).reshape(B, S, H))


# ---------------------------------------------------------------- entry point
_CACHED = {}


def _get_program():
    if "nc" not in _CACHED:
        T = FULL_B * FULL_S // N_CORES
        _CACHED["nc"] = build_program(T, FULL_H, FULL_I)
    return _CACHED["nc"]


def kernel(x, gate_packed, gate_scale, up_packed, up_scale, down_packed,
           down_scale, norm_w, _trace=False):
    from concourse.bass_utils import run_bass_kernel_spmd

    x = np.asarray(x, np.float32)
    gate_packed = np.asarray(gate_packed, np.uint8)
    up_packed = np.asarray(up_packed, np.uint8)
    down_packed = np.asarray(down_packed, np.uint8)
    norm_w = np.asarray(norm_w, np.float32)

    B, S, H = x.shape
    in_maps = prep_inputs(
        x, gate_packed, gate_scale, up_packed, up_scale, down_packed,
        down_scale, norm_w, N_CORES,
    )
    nc = _get_program()
    res = run_bass_kernel_spmd(nc, in_maps, list(range(N_CORES)), trace=_trace)
    out = assemble_output(res.results, B, S, H)
    if _trace:
        kernel.last_results = res
    return out


# revision 29
# speedup vs baseline: 1.0835x; 1.0045x over previous
"""BitNet MLP (nn_BitNetMLP_19421842112750) — TRN2 Bass kernel, 8-core
data-parallel over tokens, fp8 DoubleRow matmuls with exact hi/lo splitting.

Per core (T=1024 tokens of the 8192 total):
  G = x @ Wg_tern.T ; U = x @ Wu_tern.T
  h = relu(G)^2 * U
  var = (sum_i h^2)*A + eps ; rstd = sqrt(D / var)
  out = ((h*nw*k) @ Wd_tern.T) * rstd

Matmuls run as fp8e4 DoubleRow with zero weight-quantization error (ternary
{-1,0,1,2} is exact in e4m3) and ~9-bit effective activation mantissa: each
activation v is split into (hi, lo) = (e4m3(v), e4m3(v - hi)); the
stationary weight is broadcast (stride-0) along the DoubleRow pair dim so
each cell computes w*hi + w*lo = w*v. This matches bf16 PE throughput (the
pair doubles both MACs and MAC-rate) but halves SBUF/DMA traffic for x and
the hidden layer. x is split on the host; hidden on DVE.

Weight unpack (2-bit -> fp8e4) runs on device: DVE SWAR shift/mask then a
convert(+-1) split between ACT and DVE. Permutations pi_H / pi_I absorb the
unpack partition scramble. The ragged 64-row tail of down_packed is
re-packed host-side into one full 128-row block (shift-0/1 fields).

Device layouts:
  x8p  [H, 2, T]  fp8  rows: h = pi_H(r), pair dim = (hi, lo)
  gpTp [H/4, I]   u8   cols: i = pi_I(c)   (same for upTp)
  dpT  [IB_PAD, H] u8  natural + re-packed tail block
  nwk  [I]        f32  nwk[r] = norm_w[pi_I(r)] * k_h
  outT [H, T]     bf16 natural h rows (host transposes back)
"""

import sys

sys.path.insert(0, "/opt/trn_rl_repo")
from contextlib import ExitStack

import numpy as np
import ml_dtypes

import concourse.bass as bass
import concourse.tile as tile
from concourse import bacc, mybir

F32 = mybir.dt.float32
BF16 = mybir.dt.bfloat16
U8 = mybir.dt.uint8
U32 = mybir.dt.uint32
FP8 = mybir.dt.float8e4
SWAR_MASK = 0x03030303
AOT = mybir.AluOpType
ACTF = mybir.ActivationFunctionType
DR = mybir.MatmulPerfMode.DoubleRow
E4M3 = ml_dtypes.float8_e4m3fn
RMS_EPS = 1e-6

N_CORES = 8
FULL_B, FULL_S, FULL_H, FULL_I = 4, 2048, 2560, 6912


# ---------------------------------------------------------------- permutations
def perm_H(n):
    """SBUF row r -> original h index. Groups of 512 (4 chunks x 128)."""
    assert n % 512 == 0
    r = np.arange(n)
    c, p = r // 128, r % 128
    return 512 * (c // 4) + 128 * (p // 32) + 32 * (c % 4) + (p % 32)


def perm_I(n):
    """hidden SBUF row r -> original i index. Full 512-groups, then a
    256-tail (two 128-tiles, each split into 64-partition halves)."""
    r = np.arange(n)
    c, p = r // 128, r % 128
    out = 512 * (c // 4) + 128 * (p // 32) + 32 * (c % 4) + (p % 32)
    n_full = (n // 512) * 512
    if n_full != n:
        assert n - n_full == 256, "tail must be exactly 256"
        off = r[n_full:] - n_full
        tile_off, p2 = off // 128, off % 128
        s, q, j = p2 // 64, (p2 % 64) // 32, p2 % 32
        k = 2 * tile_off + s
        out[n_full:] = n_full + 128 * q + 32 * k + j
    return out


# ---------------------------------------------------------------- the program
def build_program(T, H, I):
    """Build the single-core Bass program (SPMD-identical across cores)."""
    NH = H // 128          # x chunks / gate-up contraction chunks (20)
    NGH = H // 512         # packed row groups per gate/up band (5)
    NI = I // 128          # hidden i-tiles (54)
    NB = NI                # gate/up bands (W_I = 128)
    TC = 512
    NT = T // TC           # 2
    IB = I // 4            # down packed rows (1728)
    NC_FULL = IB // 128    # 13
    C_TAIL = IB % 128      # 64
    IB_PAD = (NC_FULL + 1) * 128 if C_TAIL else IB
    n_cgrp = NC_FULL + (1 if C_TAIL else 0)
    assert (NH - 4) % 4 == 0
    HBANDS = [3] + [4] * ((NH - 4) // 4) + [1]
    assert sum(HBANDS) == NH

    nc = bacc.Bacc("TRN2", target_bir_lowering=False, debug=False)

    x8p = nc.dram_tensor("x8p", [H, 2, T], FP8, kind="ExternalInput").ap()
    gub = nc.dram_tensor(
        "gub", [128, NB, 2, NGH * 128], U8, kind="ExternalInput"
    ).ap()
    dpT = nc.dram_tensor("dpT", [IB_PAD, H], U8, kind="ExternalInput").ap()
    nwk = nc.dram_tensor("nwk", [I], F32, kind="ExternalInput").ap()
    a_in = nc.dram_tensor("a_in", [1], F32, kind="ExternalInput").ap()
    d_in = nc.dram_tensor("d_in", [1], F32, kind="ExternalInput").ap()
    outT = nc.dram_tensor("outT", [H, T], BF16, kind="ExternalOutput").ap()

    def bc2(w):  # stationary [K, M] -> [K, 2, M] stride-0 DoubleRow pair
        return w.unsqueeze(1).broadcast_to([w.shape[0], 2, w.shape[1]])

    with tile.TileContext(nc) as tc, ExitStack() as top:
        const = top.enter_context(tc.tile_pool(name="const", bufs=1))
        hpool = top.enter_context(tc.tile_pool(name="h8", bufs=1))
        h8 = hpool.tile([128, NI, 2, T], FP8)
        xpool = top.enter_context(tc.tile_pool(name="xT", bufs=1))
        wstage = top.enter_context(tc.tile_pool(name="wstage", bufs=5))
        ush = top.enter_context(tc.tile_pool(name="ush", bufs=1))
        wband = top.enter_context(tc.tile_pool(name="wband", bufs=2))
        dstage = top.enter_context(tc.tile_pool(name="dstage", bufs=7))
        dsh = top.enter_context(tc.tile_pool(name="dsh", bufs=3))
        wd = top.enter_context(tc.tile_pool(name="wd", bufs=3))
        outp = top.enter_context(tc.tile_pool(name="outp", bufs=3))

        # ---- gate/up band staging (DMA + SWAR unpack + fp8 convert)
        def stage_band(it):
            st = wstage.tile([128, 2, NGH * 128], U8, tag="st")
            nc.sync.dma_start(st[:], gub[:, it])
            sh = ush.tile([128, 4, 2, NGH * 128], U8, tag="sh")
            stw = st[:].bitcast(U32)
            for k in range(4):
                nc.vector.tensor_scalar(
                    sh[:, k, :, :].bitcast(U32), stw, 6 - 2 * k, SWAR_MASK,
                    AOT.logical_shift_right, AOT.bitwise_and,
                )
            wg8 = wband.tile([128, 4, NGH, 128], FP8, tag="wg")
            wu8 = wband.tile([128, 4, NGH, 128], FP8, tag="wu")
            for m, wt in ((0, wg8), (1, wu8)):
                src = sh[:, :, m, :].rearrange("p k (b w) -> p k b w", b=NGH)
                nc.scalar.activation(wt[:], src, ACTF.Copy, bias=-1.0)
            return wg8, wu8

        # first bands' weights first (so they never queue behind the x
        # stream), then x chunks
        staged_gu = {it: stage_band(it) for it in range(min(5, NB))}
        xts = []
        for c in range(NH):
            xt = xpool.tile([128, 2, T], FP8, name=f"x{c}")
            nc.sync.dma_start(xt[:], x8p[128 * c : 128 * (c + 1), :, :])
            xts.append(xt)

        # ---- constants
        a_t = const.tile([1, 1], F32)
        d_t = const.tile([1, 1], F32)
        nc.sync.dma_start(a_t[:], a_in[None, :])
        nc.sync.dma_start(d_t[:], d_in[None, :])
        nw_sb = const.tile([128, NI], F32)
        nc.sync.dma_start(nw_sb[:], nwk.rearrange("(o p) -> p o", p=128))
        s_acc = const.tile([128, T], F32)
        nc.vector.memset(s_acc[:], 0.0)
        s_bf = const.tile([128, T], BF16)
        ones_col = const.tile([128, 1], BF16)
        nc.vector.memset(ones_col[:], 1.0)
        ones_row = const.tile([1, 128], F32)
        nc.vector.memset(ones_row[:], 1.0)
        rstd_bc = const.tile([128, T], F32)

        # ---- down-proj C-group staging (DMA / unpack + convert)
        def stage_down_dma(C, h0, W):
            st = dstage.tile([128, 512], U8, tag="dst")
            nc.sync.dma_start(st[:, :W], dpT[128 * C : 128 * (C + 1), h0 : h0 + W])
            return st

        def unpack_down(st, C, W):
            tail = C >= NC_FULL
            sh4 = dsh.tile([128, 4, 512], U8, tag="dsh")
            stw = st[:, :W].bitcast(U32)
            for k in range(2) if tail else range(4):
                nc.vector.tensor_scalar(
                    sh4[:, k, :W].bitcast(U32), stw, 6 - 2 * k, SWAR_MASK,
                    AOT.logical_shift_right, AOT.bitwise_and,
                )
            wd8 = wd.tile([128, 4, 512], FP8, tag="wd8")
            kk = 2 if tail else 4
            nc.scalar.activation(
                wd8[:, 0:kk, :W], sh4[:, 0:kk, :W], ACTF.Copy, bias=-1.0
            )
            return wd8

        def stage_down(C, h0, W):
            return unpack_down(stage_down_dma(C, h0, W), C, W)

        st_pre = {}

        # ================= phase A: gate/up DoubleRow matmuls ===============
        with (
            tc.tile_pool(name="psA", bufs=2, space="PSUM") as psA,
            tc.tile_pool(name="ract", bufs=2) as ract,
        ):
            for it in range(NB):
                wg8, wu8 = staged_gu.pop(it, None) or stage_band(it)
                ps_gu = psA.tile([128, 2, T], F32, tag="pgu")
                pg = ps_gu[:, 0, :]
                pu = ps_gu[:, 1, :]
                for c in range(NH):
                    Bq, k = divmod(c, 4)
                    for ps_t, wt in ((pg, wg8), (pu, wu8)):
                        lhsT = bc2(wt[:, k, Bq, :])
                        for t in range(NT):
                            tsl = slice(TC * t, TC * (t + 1))
                            nc.tensor.matmul(
                                ps_t[:, tsl], lhsT, xts[c][:, :, tsl],
                                start=(c == 0), stop=(c == NH - 1),
                                perf_mode=DR,
                            )
                # r-stage: h = relu(G)^2 * U ; s_acc += h^2 ; h8 = split(h*nw)
                q = ract.tile([128, T], BF16, tag="q")
                h2t = ract.tile([128, T], BF16, tag="h2t")
                nc.scalar.activation(q[:], pg, ACTF.Relu)
                nc.vector.tensor_mul(q[:], q[:], q[:])
                nc.vector.tensor_mul(q[:], q[:], pu)
                nc.vector.tensor_mul(h2t[:], q[:], q[:])
                nc.vector.tensor_tensor(s_acc[:], s_acc[:], h2t[:], AOT.add)
                nwc = nw_sb[:, it : it + 1]
                nc.vector.tensor_scalar(h8[:, it, 0, :], q[:], nwc, None, AOT.mult)
                nc.vector.scalar_tensor_tensor(
                    h8[:, it, 1, :], q[:], nwc, h8[:, it, 0, :],
                    AOT.mult, AOT.subtract,
                )
                if it == NB - 1:
                    nc.vector.tensor_copy(s_bf[:], s_acc[:])
                if it == NB - 2:
                    # prefetch the first down C-groups so phase B's matmuls
                    # can start right at phase A's end (C2/C3 DMA-only)
                    W0 = HBANDS[0] * 128
                    staged_wd = [stage_down(0, 0, W0), stage_down(1, 0, W0)]
                    for Cp in range(2, min(4, n_cgrp)):
                        st_pre[(0, Cp)] = stage_down_dma(Cp, 0, W0)

        # ================= phase B + variance finalization ==================
        vpool = top.enter_context(tc.tile_pool(name="vmisc", bufs=1))
        var_sb = vpool.tile([1, T], F32)
        rv = vpool.tile([1, T], F32)
        rstd = var_sb  # reused: recip reads var_sb before Sqrt overwrites it

        def down_hband(hb_idx, h0, HB, psB, next_spec=None, pv=None, ps_bc=None):
            W = HB * 128
            po = [
                [
                    psB.tile([128, TC], F32, tag=f"po_{ht}_{t}",
                             name=f"po{hb_idx}_{ht}_{t}")
                    for t in range(NT)
                ]
                for ht in range(HB)
            ]
            for C in range(n_cgrp):
                tail = C >= NC_FULL
                ks = range(2) if tail else range(4)
                k_last = 1 if tail else 3
                if hb_idx == 0 and C < 2:
                    wd8 = staged_wd[C]
                else:
                    st = st_pre.pop((hb_idx, C), None)
                    if st is None:
                        st = stage_down_dma(C, h0, W)
                    wd8 = unpack_down(st, C, W)
                for k in ks:
                    rhs_tile = 4 * C + k
                    for ht in range(HB):
                        lhsT = bc2(wd8[:, k, 128 * ht : 128 * (ht + 1)])
                        for t in range(NT):
                            tsl = slice(TC * t, TC * (t + 1))
                            nc.tensor.matmul(
                                po[ht][t][:, :], lhsT, h8[:, rhs_tile, :, tsl],
                                start=(C == 0 and k == 0),
                                stop=(C == n_cgrp - 1 and k == k_last),
                                perf_mode=DR,
                            )
                if hb_idx == 0 and C == 1:
                    # variance colsum + rstd chain: one PSUM bank, t-serial
                    for t in range(NT):
                        tsl = slice(TC * t, TC * (t + 1))
                        nc.tensor.matmul(pv[:], ones_col[:], s_bf[:, tsl],
                                         start=True, stop=True)
                        nc.scalar.activation(var_sb[:, tsl], pv[:], ACTF.Copy,
                                             bias=RMS_EPS, scale=a_t[:])
                    nc.vector.reciprocal_approx_fast(rv[:], var_sb[:])
                    nc.scalar.activation(rstd[:], rv[:], ACTF.Sqrt, scale=d_t[:])
                if hb_idx == 0 and C == min(3, n_cgrp - 1):
                    # broadcast rstd to 128 partitions (chain done by now)
                    for t in range(NT):
                        tsl = slice(TC * t, TC * (t + 1))
                        nc.tensor.matmul(ps_bc[:], ones_row[:], rstd[:, tsl],
                                         start=True, stop=True)
                        nc.scalar.activation(rstd_bc[:, tsl], ps_bc[:], ACTF.Copy)
                if next_spec is not None and C == n_cgrp - 3:
                    nh0, nHB = next_spec
                    for Cp in range(min(4, n_cgrp)):
                        st_pre[(hb_idx + 1, Cp)] = stage_down_dma(
                            Cp, nh0, nHB * 128
                        )
            return po

        def emit_out(po, h0, HB):
            for ht in range(HB):
                for t in range(NT):
                    tsl = slice(TC * t, TC * (t + 1))
                    ot = outp.tile([128, TC], BF16, tag="ot")
                    nc.vector.tensor_mul(ot[:], po[ht][t][:], rstd_bc[:, tsl])
                    nc.sync.dma_start(
                        outT[h0 + 128 * ht : h0 + 128 * (ht + 1), tsl], ot[:]
                    )

        with tc.tile_pool(name="psV", bufs=1, space="PSUM") as psV:
            pv = psV.tile([1, TC], F32, name="pv")
            ps_bc = psV.tile([128, TC], F32, name="ps_bc")
            with tc.tile_pool(name="psB0", bufs=1, space="PSUM") as psB0:
                po0 = down_hband(0, 0, HBANDS[0], psB0,
                                 next_spec=(HBANDS[0] * 128, HBANDS[1]),
                                 pv=pv, ps_bc=ps_bc)
                emit_out(po0, 0, HBANDS[0])
        with tc.tile_pool(name="psB", bufs=1, space="PSUM") as psB:
            h0 = HBANDS[0] * 128
            for hb in range(1, len(HBANDS)):
                HB = HBANDS[hb]
                nspec = None
                if hb + 1 < len(HBANDS):
                    nspec = (h0 + HB * 128, HBANDS[hb + 1])
                po = down_hband(hb, h0, HB, psB, next_spec=nspec)
                emit_out(po, h0, HB)
                h0 += HB * 128

    nc.compile()
    return nc


# ------------------------------------------------------------- host-side prep
def unpack_host(packed, K):
    """[M, K//4] u8 -> [M, K] int8 {-1,0,1,2} (SIMD block order)."""
    M = packed.shape[0]
    b = packed.astype(np.int16).reshape(M, K // 128, 32)
    w = np.stack([(b >> 6) & 3, (b >> 4) & 3, (b >> 2) & 3, b & 3], axis=2)
    return (w.reshape(M, K) - 1).astype(np.int8)


def prep_inputs(x, gate_packed, gate_scale, up_packed, up_scale, down_packed,
                down_scale, norm_w, n_cores):
    B, S, H = x.shape
    I = norm_w.shape[0]
    T_full = B * S
    T = T_full // n_cores
    piH = perm_H(H)
    piI = perm_I(I)

    gs_v = float(np.asarray(gate_scale).reshape(-1)[0])
    us_v = float(np.asarray(up_scale).reshape(-1)[0])
    ds_v = float(np.asarray(down_scale).reshape(-1)[0])

    xf = np.ascontiguousarray(x.reshape(T_full, H), dtype=np.float32)
    # power-of-2 scale so x*s_x fits e4m3 comfortably
    s_x = 2.0 ** np.floor(np.log2(224.0 / float(np.abs(xf).max())))
    xs = xf * np.float32(s_x)
    x_hi = xs.astype(E4M3)
    x_lo = (xs - x_hi.astype(np.float32)).astype(E4M3)

    # estimate hidden absmax from a token sample to pick the fp8 range scale
    Wg_t = unpack_host(np.asarray(gate_packed, np.uint8), H)
    Wu_t = unpack_host(np.asarray(up_packed, np.uint8), H)
    idx = np.linspace(0, T_full - 1, 24).astype(np.int64)
    xr_s = x_hi[idx].astype(np.float32) + x_lo[idx].astype(np.float32)
    G_s = xr_s @ Wg_t.T.astype(np.float32)
    U_s = xr_s @ Wu_t.T.astype(np.float32)
    h_s = np.square(np.maximum(G_s, 0.0)) * U_s
    M_est = float(np.abs(h_s * norm_w[None, :].astype(np.float32)).max())
    k_h = 2.0 ** np.floor(np.log2(448.0 / (32.0 * M_est)))

    c_h = s_x**3 / (gs_v * gs_v * us_v)
    q = c_h * k_h
    A = 1.0 / (I * c_h * c_h)
    D = (ds_v / q) ** 2

    # band-contiguous gate/up packed bytes: gub[p, it, m, b*128+w] =
    # packed_m[piI[128*it + w], 128*b + p] -> one 1280B-per-partition DMA/band
    NB_, NGH_ = I // 128, H // 512
    gu = np.stack([gate_packed[piI], up_packed[piI]], axis=0)  # [2, I, H/4]
    gu = gu.reshape(2, NB_, 128, NGH_, 128)  # [m, it, w, b, p]
    gub = np.ascontiguousarray(
        gu.transpose(4, 1, 0, 3, 2).reshape(128, NB_, 2, NGH_ * 128)
    )
    dpT = np.ascontiguousarray(down_packed.T)  # [I/4, H]
    if I % 512:
        # re-pack the 64 ragged tail rows into one full 128-row block whose
        # shift-0/1 fields hold the tail i-values in h8 tile-52/53 partition
        # order (shift-2/3 fields encode weight 0)
        n_full_rows = (dpT.shape[0] // 128) * 128
        n_full_i = (I // 512) * 512
        Wd_t = unpack_host(np.asarray(down_packed, np.uint8), I)  # [H, I]
        p = np.arange(128)
        i52 = n_full_i + 128 * ((p % 64) // 32) + 32 * (p // 64) + (p % 32)
        w0 = (Wd_t[:, i52].astype(np.int16) + 1).astype(np.uint8)
        w1 = (Wd_t[:, i52 + 64].astype(np.int16) + 1).astype(np.uint8)
        pad = (w0 << 6) | (w1 << 4) | (1 << 2) | 1  # [H, 128]
        dpT = np.ascontiguousarray(
            np.concatenate([dpT[:n_full_rows], pad.T], axis=0)
        )
    nwk = np.ascontiguousarray(norm_w[piI]).astype(np.float32) * np.float32(k_h)

    in_maps = []
    for cid in range(n_cores):
        tsl = slice(cid * T, (cid + 1) * T)
        x8p = np.ascontiguousarray(
            np.stack([x_hi[tsl].T[piH], x_lo[tsl].T[piH]], axis=1)
        )  # [H, 2, T]
        in_maps.append(
            {
                "x8p": x8p,
                "gub": gub,
                "dpT": dpT,
                "nwk": nwk,
                "a_in": np.asarray([A], np.float32),
                "d_in": np.asarray([D], np.float32),
            }
        )
    return in_maps


def assemble_output(results, B, S, H):
    outs = [np.asarray(r["outT"]).astype(np.float32).T for r in results]
    return np.ascontiguousarray(np.concatenate(outs, axis=0).reshape(B, S, H))


# ---------------------------------------------------------------- entry point
_CACHED = {}


def _get_program():
    if "nc" not in _CACHED:
        T = FULL_B * FULL_S // N_CORES
        _CACHED["nc"] = build_program(T, FULL_H, FULL_I)
    return _CACHED["nc"]


def kernel(x, gate_packed, gate_scale, up_packed, up_scale, down_packed,
           down_scale, norm_w, _trace=False):
    from concourse.bass_utils import run_bass_kernel_spmd

    x = np.asarray(x, np.float32)
    gate_packed = np.asarray(gate_packed, np.uint8)
    up_packed = np.asarray(up_packed, np.uint8)
    down_packed = np.asarray(down_packed, np.uint8)
    norm_w = np.asarray(norm_w, np.float32)

    B, S, H = x.shape
    in_maps = prep_inputs(
        x, gate_packed, gate_scale, up_packed, up_scale, down_packed,
        down_scale, norm_w, N_CORES,
    )
    nc = _get_program()
    res = run_bass_kernel_spmd(nc, in_maps, list(range(N_CORES)), trace=_trace)
    out = assemble_output(res.results, B, S, H)
    if _trace:
        kernel.last_results = res
    return out
